# revision 46
# baseline (speedup 1.0000x reference)
"""Multi-head causal attention (B=2, S=2048, D=1024, H=16, Dh=64) on 8 TRN2 cores.

Sharding: tensor-parallel over 4 head-groups x data-parallel over 2 batches.
Core c handles batch c//4, heads [4*(c%4), 4*(c%4)+4). Each core computes its
partial output projection; the host sums the 4 partials per batch (the
"all-reduce") and adds b_O.

Mixed-precision device program (fp32 PSUM accumulation everywhere):
  - Q/K/V projections: fp8e4m3 DoubleRow matmuls (0.5 cyc/row, 256-deep
    contraction per instr). Host pre-splits x and 64*W into hi/lo fp8 pairs;
    3 compensation terms xh@Wh + xl@Wh + xh@Wl recover ~bf16 accuracy.
  - Scores S = (64q)(64k): bf16 QT/KT, per-128-column causal crops. The /8
    softmax scale and the 1/4096 fp8 weight scaling fold into the ACT exp
    (scale=1/32768).
  - P = exp(S): ACT writes fp8 tiles for off-diagonal k-pairs, bf16 for the
    4 diagonal k-tiles (where P concentrates and fp8 noise would not average
    out). Triangular masks multiply bf16 diag tiles only (DVE 2x on bf16).
  - Z = P @ [64V|1]: off-diag via fp8 DoubleRow over k-tile pairs (0.25
    cyc/row/k-tile), diag via bf16-moving matmuls; row 64 accumulates the
    softmax denominator.
  - out += (Z/denom)^T @ (Wo/64): f32r stationary ZT, bf16 moving WO.
"""

import numpy as np
import ml_dtypes

import concourse.mybir as mybir
import concourse.tile as tile
from concourse import bacc
from concourse import bass_utils

F32 = mybir.dt.float32
F32R = mybir.dt.float32r
BF16 = mybir.dt.bfloat16
F8 = mybir.dt.float8e4

SEQ = 2048
DM = 1024
DH = 64
HLOC = 4          # heads per core
KCP = 4           # dmodel pair-chunks of 256
NQC = 4           # q chunks of 512
QW = 512
WS = 64.0         # fp8 weight scale
EXP_SCALE = 1.0 / (WS * WS * 8.0)
DR = mybir.MatmulPerfMode.DoubleRow

_PROGRAMS = {}


def _build(with_bias: bool):
    nc = bacc.Bacc("TRN2", target_bir_lowering=False, debug=False, num_devices=8)

    # [128, j(4), i(2), qb(4), 512] with element [p,j,i,qb,s] = xT[256j+128i+p,
    # 512qb+s]; hi/lo fp8 split of xT
    xh8 = nc.dram_tensor("xh8", [128, KCP, 2, NQC, QW], F8, kind="ExternalInput").ap()
    xl8 = nc.dram_tensor("xl8", [128, KCP, 2, NQC, QW], F8, kind="ExternalInput").ap()
    # [128, j(4), i(2), 256]: hi/lo of 64*W[256j+128i+p, m]
    wname = lambda n: nc.dram_tensor(n, [128, KCP, 2, 256], F8, kind="ExternalInput").ap()
    wqh, wql = wname("wqh"), wname("wql")
    wkh, wkl = wname("wkh"), wname("wkl")
    wvh, wvl = wname("wvh"), wname("wvl")
    wo = nc.dram_tensor("wo", [256, DM], BF16, kind="ExternalInput").ap()
    z8 = nc.dram_tensor("z8", [128, 2048], F8, kind="ExternalInput").ap()
    tri = nc.dram_tensor("tri", [128, 128], BF16, kind="ExternalInput").ap()
    if with_bias:
        bqkv = nc.dram_tensor("bqkv", [1, 768], BF16, kind="ExternalInput").ap()
        ones = nc.dram_tensor("ones", [1, SEQ], BF16, kind="ExternalInput").ap()
    out = nc.dram_tensor("out", [SEQ, DM], F32, kind="ExternalOutput").ap()

    with tile.TileContext(nc) as tc:
        with (
            tc.tile_pool(name="px", bufs=1) as px,
            tc.tile_pool(name="pw", bufs=1) as pw,
            tc.tile_pool(name="pqk", bufs=1) as pqk,
            tc.tile_pool(name="pv", bufs=1) as pv,
            tc.tile_pool(name="ppt8", bufs=4) as ppt8,
            tc.tile_pool(name="pptb", bufs=4) as pptb,
            tc.tile_pool(name="pzt", bufs=4) as pzt,
            tc.tile_pool(name="prs", bufs=3) as prs,
            tc.tile_pool(name="pout", bufs=4) as pout,
            tc.tile_pool(name="psS", bufs=2, space="PSUM") as psS,
            tc.tile_pool(name="psZ", bufs=2, space="PSUM") as psZ,
            tc.tile_pool(name="psF", bufs=2, space="PSUM") as psF,
        ):
            # ---- SBUF tiles ----
            XH = px.tile([128, KCP, 2, NQC, QW], F8, tag="xh")
            XL = px.tile([128, KCP, 2, NQC, QW], F8, tag="xl")
            WQh = pw.tile([128, KCP, 2, 256], F8, tag="wqh")
            WQl = pw.tile([128, KCP, 2, 256], F8, tag="wql")
            WKh = pw.tile([128, KCP, 2, 256], F8, tag="wkh")
            WKl = pw.tile([128, KCP, 2, 256], F8, tag="wkl")
            WVh = pw.tile([128, KCP, 2, 256], F8, tag="wvh")
            WVl = pw.tile([128, KCP, 2, 256], F8, tag="wvl")

            QT = [pqk.tile([128, NQC, 2, QW], F8, tag=f"qt{hp}", name=f"qt{hp}")
                  for hp in range(2)]
            KT = [pqk.tile([128, 16, 2, 128], F8, tag=f"kt{hp}", name=f"kt{hp}")
                  for hp in range(2)]

            # ---- input DMA: qb0 x-blocks + QK weights first ----
            nc.sync.dma_start(WQh[:], wqh)
            nc.sync.dma_start(WQl[:], wql)
            nc.sync.dma_start(XH[:, 0, :, 0, :], xh8[:, 0, :, 0, :])
            nc.sync.dma_start(XL[:, 0, :, 0, :], xl8[:, 0, :, 0, :])
            nc.sync.dma_start(XH[:, 1:4, :, 0, :], xh8[:, 1:4, :, 0, :])
            nc.sync.dma_start(XL[:, 1:4, :, 0, :], xl8[:, 1:4, :, 0, :])
            nc.sync.dma_start(WKh[:], wkh)
            nc.sync.dma_start(WKl[:], wkl)
            for hp in range(2):
                nc.sync.dma_start(
                    QT[hp][:, :, 1, :],
                    z8[:, 0:NQC * QW].rearrange("p (a b) -> p a b", a=NQC))
                nc.sync.dma_start(
                    KT[hp][:, :, 1, :],
                    z8[:, 0:2048].rearrange("p (a b) -> p a b", a=16))
            nc.sync.dma_start(WVh[:], wvh)
            nc.sync.dma_start(WVl[:], wvl)
            TRI = pw.tile([128, 128], BF16, tag="tri")
            nc.sync.dma_start(TRI[:], tri)
            nc.sync.dma_start(XH[:, :, :, 1, :], xh8[:, :, :, 1, :])
            nc.sync.dma_start(XL[:, :, :, 1, :], xl8[:, :, :, 1, :])
            WO = pw.tile([128, 2, DM], BF16, tag="wo")
            for hp in range(2):
                nc.sync.dma_start(WO[:, hp, :], wo[hp * 128:(hp + 1) * 128, :])
            for qb in range(2, NQC):
                nc.sync.dma_start(XH[:, :, :, qb, :], xh8[:, :, :, qb, :])
                nc.sync.dma_start(XL[:, :, :, qb, :], xl8[:, :, :, qb, :])
            if with_bias:
                BQKV = pw.tile([1, 768], BF16, tag="bqkv")
                nc.sync.dma_start(BQKV[:], bqkv)
                ONES = pw.tile([1, SEQ], BF16, tag="ones")
                nc.sync.dma_start(ONES[:], ones)

            # QT/KT fp8 with an interleaved zero slot: scores run as
            # zero-padded DoubleRow matmuls at 0.5 cyc/row. Slot 1 of both
            # stationary and moving operands is zeroed via DMA so the second
            # dual-row tile contributes exactly 0.
            VB = pv.tile([128, 16, HLOC, DH + 1], BF16, tag="vb")
            # innermost padded to 68 so the DoubleRow pair stride (4*68)
            # is 16-element aligned (s3_lw_dual_fp8 ISA restriction)
            V8 = pv.tile([128, 6, 2, HLOC, DH + 4], F8, tag="v8")
            nc.vector.memset(VB[:, :, :, DH:DH + 1], 1.0)
            nc.vector.tensor_copy(
                V8[:, :, :, :, DH:DH + 1].rearrange("p a b h o -> p (a b) h o"),
                VB[:, 0:12, :, DH:DH + 1],
            )

            # ---- projections: fp8 DoubleRow 3-term chains (generators
            # yielding after each matmul so fillers can interleave) ----
            def qk_chain_g(pp_slice, w_hi, w_lo, hp, qc, bias_off):
                for j in range(KCP):
                    for term, (w_sb, x_sb) in enumerate(
                        ((w_hi, XH), (w_hi, XL), (w_lo, XH))
                    ):
                        nc.tensor.matmul(
                            pp_slice,
                            w_sb[:, j, :, hp * 128:(hp + 1) * 128],
                            x_sb[:, j, :, qc, :],
                            start=(j == 0 and term == 0),
                            stop=(j == KCP - 1 and term == 2 and not with_bias),
                            perf_mode=DR,
                        )
                        yield
                if with_bias:
                    nc.tensor.matmul(
                        pp_slice,
                        BQKV[0:1, bias_off + hp * 128:bias_off + (hp + 1) * 128],
                        ONES[0:1, qc * QW:(qc + 1) * QW],
                        start=False, stop=True,
                    )
                    yield

            def v_chain_g(pp_slice, st):
                qb, s0 = st // 4, (st % 4) * 128
                for j in range(KCP):
                    for term, (x_sb, w_sb) in enumerate(
                        ((XH, WVh), (XL, WVh), (XH, WVl))
                    ):
                        nc.tensor.matmul(
                            pp_slice,
                            x_sb[:, j, :, qb, s0:s0 + 128],
                            w_sb[:, j, :, :],
                            start=(j == 0 and term == 0),
                            stop=(j == KCP - 1 and term == 2 and not with_bias),
                            perf_mode=DR,
                        )
                        yield
                if with_bias:
                    nc.tensor.matmul(
                        pp_slice,
                        ONES[0:1, st * 128:(st + 1) * 128],
                        BQKV[0:1, 512:768],
                        start=False, stop=True,
                    )
                    yield

            class Filler:
                """FIFO of PE micro-work generators; m-loops pull a few
                matmuls at a time so proj/O-proj work interleaves with the
                ACT-paced attention stream."""

                def __init__(self):
                    self.units = []
                    self.cur = None
                    self.cur_marker = None

                def add(self, marker, genfn):
                    self.units.append((marker, genfn))

                def pull(self, n):
                    while n > 0:
                        if self.cur is None:
                            if not self.units:
                                return
                            self.cur_marker, genfn = self.units.pop(0)
                            self.cur = genfn()
                        try:
                            next(self.cur)
                            n -= 1
                        except StopIteration:
                            self.cur = None

                def drain_marker(self, marker):
                    while (self.cur is not None and self.cur_marker == marker) \
                            or any(m == marker for m, _ in self.units):
                        self.pull(1)

            # ---- projection chain-tile emitters (PE filler units) ----
            def qk_single_g(qc, hp, which):
                # Q/K chain as a psF single, generator form
                w_hi, w_lo, dst, boff = (
                    (WQh, WQl, QT, 0) if which == "q" else (WKh, WKl, KT, 256))
                pz = psF.tile([128, QW], F32, tag="f",
                              name=f"f{which}{qc}_{hp}")
                yield from qk_chain_g(pz[:], w_hi, w_lo, hp, qc, boff)
                if which == "q":
                    nc.vector.tensor_copy(dst[hp][:, qc, 0, :], pz[:])
                else:
                    nc.vector.tensor_copy(
                        dst[hp][:, 4 * qc:4 * qc + 4, 0, :],
                        pz[:].rearrange("p (k c) -> p k c", k=4))

            def v_single_g(st):
                pz = psF.tile([128, QW], F32, tag="f", name=f"fv{st}")
                yield from v_chain_g(pz[:, 0:256], st)
                nc.vector.tensor_copy(
                    VB[:, st, :, 0:DH],
                    pz[:, 0:256].rearrange("p (h d) -> p h d", h=HLOC),
                )
                if st < 12:
                    nc.vector.tensor_copy(
                        V8[:, st // 2, st % 2, :, 0:DH],
                        pz[:, 0:256].rearrange("p (h d) -> p h d", h=HLOC),
                    )

            def emit_qk_pair(qc, which):
                w_hi, w_lo, dst, boff = (
                    (WQh, WQl, QT, 0) if which == "q" else (WKh, WKl, KT, 256))
                pp = psS.tile([128, 2, QW], F32, tag="s", name=f"{which}p{qc}")
                for hp in range(2):
                    for _ in qk_chain_g(pp[:, hp, :], w_hi, w_lo, hp, qc, boff):
                        pass
                for hp in range(2):
                    if which == "q":
                        nc.vector.tensor_copy(dst[hp][:, qc, 0, :], pp[:, hp, :])
                    else:
                        nc.vector.tensor_copy(
                            dst[hp][:, 4 * qc:4 * qc + 4, 0, :],
                            pp[:, hp, :].rearrange("p (k c) -> p k c", k=4))

            def emit_v_pair(st2):
                pp = psS.tile([128, 2, QW], F32, tag="s", name=f"vp{st2}")
                for i in range(2):
                    for _ in v_chain_g(pp[:, i, 0:256], st2 * 2 + i):
                        pass
                for i in range(2):
                    st = st2 * 2 + i
                    nc.vector.tensor_copy(
                        VB[:, st, :, 0:DH],
                        pp[:, i, 0:256].rearrange("p (h d) -> p h d", h=HLOC),
                    )
                    if st < 12:
                        nc.vector.tensor_copy(
                            V8[:, st // 2, st % 2, :, 0:DH],
                            pp[:, i, 0:256].rearrange("p (h d) -> p h d", h=HLOC),
                        )

            # ---- output projection units (PE filler; one (qt,mc) each) ----
            ZTS = {}   # qc -> [ZT_hp0, ZT_hp1]
            OSB = {}   # (qc, qt) -> staging tile; DMA fires after mc=1

            def o_single_g(qc, qt, mc):
                q0 = qc * QW
                ZT = ZTS[qc]
                t = psF.tile([128, QW], F32, tag="f", name=f"o{qc}_{qt}_{mc}")
                if mc == 0:
                    OSB[(qc, qt)] = pout.tile([128, DM], F32, tag="ob",
                                              name=f"ob{qc}_{qt}")
                osb = OSB[(qc, qt)]
                for hp in range(2):
                    nc.tensor.matmul(
                        t[:],
                        ZT[hp][:, qt * 128:(qt + 1) * 128],
                        WO[:, hp, mc * QW:(mc + 1) * QW],
                        start=(hp == 0), stop=(hp == 1),
                    )
                    yield
                if qc == 3 and (qt + mc) % 2 == 1:
                    # tail: ACT is idle after the last exp; splitting the
                    # final copies across DVE+ACT halves the drain
                    nc.scalar.copy(osb[:, mc * QW:(mc + 1) * QW], t[:])
                else:
                    nc.vector.tensor_copy(osb[:, mc * QW:(mc + 1) * QW], t[:])
                if mc == 1:
                    nc.sync.dma_start(
                        out[q0 + qt * 128:q0 + (qt + 1) * 128, :], osb[:])

            def emit_o_unit(qc, qt):
                for mc in range(2):
                    for _ in o_single_g(qc, qt, mc):
                        pass

            # ---- one attention head: scores->exp->Z with the Z matmuls
            # delayed one s-tile behind (exp latency hiding) and filler
            # matmuls pulled between iterations ----
            norm_pend = []   # deferred (bcast-mm + ZT-mul) closures

            def drain_norm():
                while norm_pend:
                    norm_pend.pop(0)()

            def emit_head(qc, h, filler):
                hp, hh = h // 2, h % 2
                hs = slice(hh * DH, (hh + 1) * DH)
                zps = psZ.tile([128, QW], F32, tag="z", name=f"z{qc}_{h}")

                def s_mm(dst, kt, q_lo, q_hi):
                    nc.tensor.matmul(
                        dst,
                        KT[hp][hs, kt, :, :],
                        QT[hp][hs, qc, :, q_lo:q_hi],
                        start=True, stop=True,
                        perf_mode=DR,
                    )

                def z_dr(m, pt8):
                    nc.tensor.matmul(
                        zps[0:DH + 1, :], V8[:, m, :, h, 0:DH + 1], pt8[:],
                        start=(m == 0), stop=False,
                        perf_mode=DR, skip_group_check=True,
                    )

                pend = []
                for m in range(2 * qc):
                    sps = psS.tile([128, 2, QW], F32, tag="s",
                                   name=f"s{qc}_{h}_{m}")
                    for i in range(2):
                        s_mm(sps[:, i, :], 2 * m + i, 0, QW)
                    drain_norm()  # deferred prev-head normalize on PE slack
                    pt8 = ppt8.tile([128, 2, QW], F8, tag="p8",
                                    name=f"p8_{qc}_{h}_{m}")
                    nc.scalar.activation(
                        pt8[:], sps[:], mybir.ActivationFunctionType.Exp,
                        scale=EXP_SCALE,
                    )
                    pend.append((z_dr, m, pt8))
                    filler.pull(4)
                    if len(pend) > 1:
                        fn, mm, pt = pend.pop(0)
                        fn(mm, pt)

                ktA, ktB = 4 * qc, 4 * qc + 2
                sA = psS.tile([128, 2, QW], F32, tag="s", name=f"sA{qc}_{h}")
                s_mm(sA[:, 0, :], ktA, 0, QW)
                s_mm(sA[:, 1, :], ktA + 1, 0, QW)
                drain_norm()
                ptA = pptb.tile([128, 2, QW], BF16, tag="pb", name=f"pA{qc}_{h}")
                nc.scalar.activation(
                    ptA[:], sA[:], mybir.ActivationFunctionType.Exp,
                    scale=EXP_SCALE,
                )
                for fn, mm, pt in pend:
                    fn(mm, pt)
                sB = psS.tile([128, 2, QW], F32, tag="s", name=f"sB{qc}_{h}")
                s_mm(sB[:, 0, 0:256], ktB, 256, QW)
                s_mm(sB[:, 0, 256:QW], ktB + 1, 256, QW)
                ptB = pptb.tile([128, 2, QW], BF16, tag="pb", name=f"pB{qc}_{h}")
                nc.scalar.activation(
                    ptB[:, 0, :], sB[:, 0, :],
                    mybir.ActivationFunctionType.Exp, scale=EXP_SCALE,
                )
                nc.gpsimd.tensor_mul(ptA[:, 0, 0:128], ptA[:, 0, 0:128], TRI[:])
                nc.gpsimd.tensor_mul(ptA[:, 1, 128:256], ptA[:, 1, 128:256], TRI[:])
                nc.gpsimd.tensor_mul(ptB[:, 0, 0:128], ptB[:, 0, 0:128], TRI[:])
                nc.gpsimd.tensor_mul(ptB[:, 0, 384:QW], ptB[:, 0, 384:QW], TRI[:])

                # PE filler while the diagonal exps+masks complete
                if qc == 0 and h == 0:
                    filler.drain_marker("pv0")  # VB st0-3 for the diag Z
                else:
                    filler.pull(6)

                def z_bf(v_st, pt_ap, q_lo, q_hi, start, stop):
                    nc.tensor.matmul(
                        zps[0:DH + 1, q_lo:q_hi],
                        VB[:, v_st, h, :], pt_ap,
                        start=start, stop=stop, skip_group_check=True,
                    )

                z_bf(ktA, ptA[:, 0, :], 0, QW, qc == 0, False)
                z_bf(ktA + 1, ptA[:, 1, 128:QW], 128, QW, False, False)
                z_bf(ktB, ptB[:, 0, 0:256], 256, QW, False, False)
                z_bf(ktB + 1, ptB[:, 0, 384:QW], 384, QW, False, True)

                recip = prs.tile([1, QW], F32R, tag="recip", name=f"rc{qc}_{h}")
                with nc.allow_low_precision(reason="softmax recip in fp32r"):
                    nc.vector.reciprocal(recip[:], zps[DH:DH + 1, :])
                rb = prs.tile([DH, QW], F32R, tag="rb", name=f"rb{qc}_{h}")
                nc.gpsimd.partition_broadcast(rb[:], recip[:])
                if qc not in ZTS:
                    ZTS[qc] = [
                        pzt.tile([128, QW], BF16, tag="zt", name=f"zt{qc}_{p}")
                        for p in range(2)
                    ]
                nc.vector.tensor_mul(ZTS[qc][hp][hs, :], zps[0:DH, :], rb[:])

            # ---- global pipelined emission ----
            # minimal prefix so the first exp fires early: Q-hp0, K pair,
            # V pairs for qc0 diag; everything else rides the filler queue
            for _ in qk_single_g(0, 0, "q"):
                pass
            emit_qk_pair(0, "k")

            filler = Filler()
            for st in range(4):
                filler.add("pv0", lambda st=st: v_single_g(st))
            filler.add("p0", lambda: qk_single_g(0, 1, "q"))
            for qcn in (1, 2, 3):
                for which in ("q", "k"):
                    for hp in range(2):
                        filler.add(f"p{qcn}",
                                   (lambda qcn=qcn, hp=hp, which=which:
                                    qk_single_g(qcn, hp, which)))
                for st in range(4 * qcn, 4 * qcn + 4):
                    filler.add(f"p{qcn}", lambda st=st: v_single_g(st))
            def add_o_units(qcp):
                # added only after head(qcp,3) is emitted so no O matmul can
                # be pulled ahead of its ZT writes
                for qt in range(4):
                    for mc in range(2):
                        filler.add(f"o{qcp}",
                                   (lambda qcp=qcp, qt=qt, mc=mc:
                                    o_single_g(qcp, qt, mc)))

            # head order: qc-major, but (3,0) hoisted before (2,3) so qc3's
            # ACT work starts earlier
            order = [(0, 0), (0, 1), (0, 2), (0, 3),
                     (1, 0), (1, 1), (1, 2), (1, 3),
                     (2, 0), (2, 1), (2, 2), (3, 0), (2, 3),
                     (3, 1), (3, 2), (3, 3)]
            for qc, h in order:
                if h == 0 and qc > 0:
                    filler.drain_marker(f"p{qc}")
                if qc == 0 and h == 2:
                    filler.drain_marker("p0")  # QT hp1 needed
                emit_head(qc, h, filler)
                if h == 3 and qc < 3:
                    add_o_units(qc)
            drain_norm()
            filler.drain_marker("o2")
            for qt in range(4):
                emit_o_unit(3, qt)

    nc.compile()
    return nc


def _get_program(with_bias: bool):
    if with_bias not in _PROGRAMS:
        _PROGRAMS[with_bias] = _build(with_bias)
    return _PROGRAMS[with_bias]


def _split8(a):
    hi = a.astype(ml_dtypes.float8_e4m3)
    lo = (a - hi.astype(np.float32)).astype(ml_dtypes.float8_e4m3)
    return hi, lo


def _x_layout(a):
    # [1024, 2048] -> [128, j, i, qb, s]
    return np.ascontiguousarray(
        a.reshape(KCP, 2, 128, NQC, QW).transpose(2, 0, 1, 3, 4))


def _w_layout(a):
    # [1024, 256] -> [128, j, i, 256]
    return np.ascontiguousarray(
        a.reshape(KCP, 2, 128, 256).transpose(2, 0, 1, 3))


def kernel(normalized_resid_pre, W_Q, W_K, W_V, W_O, b_Q, b_K, b_V, b_O):
    x = np.asarray(normalized_resid_pre, dtype=np.float32)
    W_Q = np.asarray(W_Q, dtype=np.float32)
    W_K = np.asarray(W_K, dtype=np.float32)
    W_V = np.asarray(W_V, dtype=np.float32)
    W_O = np.asarray(W_O, dtype=np.float32)
    b_Q = np.asarray(b_Q, dtype=np.float32)
    b_K = np.asarray(b_K, dtype=np.float32)
    b_V = np.asarray(b_V, dtype=np.float32)
    b_O = np.asarray(b_O, dtype=np.float32)

    batch, seq, dm = x.shape
    with_bias = bool(np.any(b_Q) or np.any(b_K) or np.any(b_V))
    nc = _get_program(with_bias)

    tri = np.ascontiguousarray(
        np.triu(np.ones((128, 128), np.float32)).astype(ml_dtypes.bfloat16))

    xsp = []
    for b in range(batch):
        xh, xl = _split8(np.ascontiguousarray(x[b].T))
        xsp.append((_x_layout(xh), _x_layout(xl)))

    in_maps = []
    for c in range(8):
        b, g = c // 4, c % 4
        hs = slice(4 * g, 4 * g + 4)
        m = {"xh8": xsp[b][0], "xl8": xsp[b][1], "tri": tri,
             "z8": np.zeros((128, 2048), ml_dtypes.float8_e4m3)}
        for nm, W in (("wq", W_Q), ("wk", W_K), ("wv", W_V)):
            Wp = np.transpose(W[hs], (1, 0, 2)).reshape(dm, 256) * WS
            hi, lo = _split8(Wp)
            m[nm + "h"] = _w_layout(hi)
            m[nm + "l"] = _w_layout(lo)
        m["wo"] = np.ascontiguousarray(
            (W_O[hs].reshape(256, dm) / WS).astype(ml_dtypes.bfloat16))
        if with_bias:
            m["bqkv"] = np.ascontiguousarray(np.concatenate(
                [b_Q[hs].reshape(256) * WS, b_K[hs].reshape(256) * WS,
                 b_V[hs].reshape(256) * WS]
            )[None, :].astype(ml_dtypes.bfloat16))
            m["ones"] = np.ones((1, seq), ml_dtypes.bfloat16)
        in_maps.append(m)

    res = bass_utils.run_bass_kernel_spmd(nc, in_maps, core_ids=list(range(8)))
    parts = [res.results[c]["out"] for c in range(8)]
    full = np.stack(
        [parts[0] + parts[1] + parts[2] + parts[3],
         parts[4] + parts[5] + parts[6] + parts[7]]
    )
    full += b_O
    return full.astype(np.float32)


# revision 50
# speedup vs baseline: 1.0128x; 1.0128x over previous
"""Multi-head causal attention (B=2, S=2048, D=1024, H=16, Dh=64) on 8 TRN2 cores.

Sharding: tensor-parallel over 4 head-groups x data-parallel over 2 batches.
Core c handles batch c//4, heads [4*(c%4), 4*(c%4)+4). Each core computes its
partial output projection; the host sums the 4 partials per batch (the
"all-reduce") and adds b_O.

Mixed-precision device program (fp32 PSUM accumulation everywhere):
  - Q/K/V projections: fp8e4m3 DoubleRow matmuls (0.5 cyc/row, 256-deep
    contraction per instr). Host pre-splits x and 64*W into hi/lo fp8 pairs;
    3 compensation terms xh@Wh + xl@Wh + xh@Wl recover ~bf16 accuracy.
  - Scores S = (64q)(64k): bf16 QT/KT, per-128-column causal crops. The /8
    softmax scale and the 1/4096 fp8 weight scaling fold into the ACT exp
    (scale=1/32768).
  - P = exp(S): ACT writes fp8 tiles for off-diagonal k-pairs, bf16 for the
    4 diagonal k-tiles (where P concentrates and fp8 noise would not average
    out). Triangular masks multiply bf16 diag tiles only (DVE 2x on bf16).
  - Z = P @ [64V|1]: off-diag via fp8 DoubleRow over k-tile pairs (0.25
    cyc/row/k-tile), diag via bf16-moving matmuls; row 64 accumulates the
    softmax denominator.
  - out += (Z/denom)^T @ (Wo/64): f32r stationary ZT, bf16 moving WO.
"""

import numpy as np
import ml_dtypes

import concourse.mybir as mybir
import concourse.tile as tile
from concourse import bacc
from concourse import bass_utils

F32 = mybir.dt.float32
F32R = mybir.dt.float32r
BF16 = mybir.dt.bfloat16
F8 = mybir.dt.float8e4

SEQ = 2048
DM = 1024
DH = 64
HLOC = 4          # heads per core
KCP = 4           # dmodel pair-chunks of 256
NQC = 4           # q chunks of 512
QW = 512
WS = 64.0         # fp8 weight scale
EXP_SCALE = 1.0 / (WS * WS * 8.0)
DR = mybir.MatmulPerfMode.DoubleRow

_PROGRAMS = {}


def _build(with_bias: bool):
    nc = bacc.Bacc("TRN2", target_bir_lowering=False, debug=False, num_devices=8)

    # [128, j(4), i(2), qb(4), 512] with element [p,j,i,qb,s] = xT[256j+128i+p,
    # 512qb+s]; hi/lo fp8 split of xT
    xh8 = nc.dram_tensor("xh8", [128, KCP, 2, NQC, QW], F8, kind="ExternalInput").ap()
    xl8 = nc.dram_tensor("xl8", [128, KCP, 2, NQC, QW], F8, kind="ExternalInput").ap()
    # [128, j(4), i(2), 256]: hi/lo of 64*W[256j+128i+p, m]
    wname = lambda n: nc.dram_tensor(n, [128, KCP, 2, 256], F8, kind="ExternalInput").ap()
    wqh, wql = wname("wqh"), wname("wql")
    wkh, wkl = wname("wkh"), wname("wkl")
    wvh, wvl = wname("wvh"), wname("wvl")
    wo = nc.dram_tensor("wo", [256, DM], BF16, kind="ExternalInput").ap()
    z8 = nc.dram_tensor("z8", [128, 2048], F8, kind="ExternalInput").ap()
    tri = nc.dram_tensor("tri", [128, 128], BF16, kind="ExternalInput").ap()
    if with_bias:
        bqkv = nc.dram_tensor("bqkv", [1, 768], BF16, kind="ExternalInput").ap()
        ones = nc.dram_tensor("ones", [1, SEQ], BF16, kind="ExternalInput").ap()
    out = nc.dram_tensor("out", [SEQ, DM], F32, kind="ExternalOutput").ap()

    with tile.TileContext(nc) as tc:
        with (
            tc.tile_pool(name="px", bufs=1) as px,
            tc.tile_pool(name="pw", bufs=1) as pw,
            tc.tile_pool(name="pqk", bufs=1) as pqk,
            tc.tile_pool(name="pv", bufs=1) as pv,
            tc.tile_pool(name="ppt8", bufs=4) as ppt8,
            tc.tile_pool(name="pptb", bufs=4) as pptb,
            tc.tile_pool(name="pzt", bufs=4) as pzt,
            tc.tile_pool(name="prs", bufs=3) as prs,
            tc.tile_pool(name="pout", bufs=4) as pout,
            tc.tile_pool(name="psS", bufs=2, space="PSUM") as psS,
            tc.tile_pool(name="psZ", bufs=2, space="PSUM") as psZ,
            tc.tile_pool(name="psF", bufs=2, space="PSUM") as psF,
        ):
            # ---- SBUF tiles ----
            XH = px.tile([128, KCP, 2, NQC, QW], F8, tag="xh")
            XL = px.tile([128, KCP, 2, NQC, QW], F8, tag="xl")
            WQh = pw.tile([128, KCP, 2, 256], F8, tag="wqh")
            WQl = pw.tile([128, KCP, 2, 256], F8, tag="wql")
            WKh = pw.tile([128, KCP, 2, 256], F8, tag="wkh")
            WKl = pw.tile([128, KCP, 2, 256], F8, tag="wkl")
            WVh = pw.tile([128, KCP, 2, 256], F8, tag="wvh")
            WVl = pw.tile([128, KCP, 2, 256], F8, tag="wvl")

            QT = [pqk.tile([128, NQC, 2, QW], F8, tag=f"qt{hp}", name=f"qt{hp}")
                  for hp in range(2)]
            KT = [pqk.tile([128, 16, 2, 128], F8, tag=f"kt{hp}", name=f"kt{hp}")
                  for hp in range(2)]

            # ---- input DMA: qb0 x-blocks + QK weights first ----
            nc.sync.dma_start(WQh[:], wqh)
            nc.sync.dma_start(WQl[:], wql)
            nc.sync.dma_start(XH[:, 0, :, 0, :], xh8[:, 0, :, 0, :])
            nc.sync.dma_start(XL[:, 0, :, 0, :], xl8[:, 0, :, 0, :])
            nc.sync.dma_start(XH[:, 1:4, :, 0, :], xh8[:, 1:4, :, 0, :])
            nc.sync.dma_start(XL[:, 1:4, :, 0, :], xl8[:, 1:4, :, 0, :])
            nc.sync.dma_start(WKh[:], wkh)
            nc.sync.dma_start(WKl[:], wkl)
            for hp in range(2):
                nc.sync.dma_start(
                    QT[hp][:, :, 1, :],
                    z8[:, 0:NQC * QW].rearrange("p (a b) -> p a b", a=NQC))
                nc.sync.dma_start(
                    KT[hp][:, :, 1, :],
                    z8[:, 0:2048].rearrange("p (a b) -> p a b", a=16))
            nc.sync.dma_start(WVh[:], wvh)
            nc.sync.dma_start(WVl[:], wvl)
            TRI = pw.tile([128, 128], BF16, tag="tri")
            nc.sync.dma_start(TRI[:], tri)
            nc.sync.dma_start(XH[:, :, :, 1, :], xh8[:, :, :, 1, :])
            nc.sync.dma_start(XL[:, :, :, 1, :], xl8[:, :, :, 1, :])
            WO = pw.tile([128, 2, DM], BF16, tag="wo")
            for hp in range(2):
                nc.sync.dma_start(WO[:, hp, :], wo[hp * 128:(hp + 1) * 128, :])
            for qb in range(2, NQC):
                nc.sync.dma_start(XH[:, :, :, qb, :], xh8[:, :, :, qb, :])
                nc.sync.dma_start(XL[:, :, :, qb, :], xl8[:, :, :, qb, :])
            if with_bias:
                BQKV = pw.tile([1, 768], BF16, tag="bqkv")
                nc.sync.dma_start(BQKV[:], bqkv)
                ONES = pw.tile([1, SEQ], BF16, tag="ones")
                nc.sync.dma_start(ONES[:], ones)

            # QT/KT fp8 with an interleaved zero slot: scores run as
            # zero-padded DoubleRow matmuls at 0.5 cyc/row. Slot 1 of both
            # stationary and moving operands is zeroed via DMA so the second
            # dual-row tile contributes exactly 0.
            VB = pv.tile([128, 16, HLOC, DH + 1], BF16, tag="vb")
            # innermost padded to 68 so the DoubleRow pair stride (4*68)
            # is 16-element aligned (s3_lw_dual_fp8 ISA restriction)
            V8 = pv.tile([128, 6, 2, HLOC, DH + 4], F8, tag="v8")
            nc.vector.memset(VB[:, :, :, DH:DH + 1], 1.0)
            nc.vector.tensor_copy(
                V8[:, :, :, :, DH:DH + 1].rearrange("p a b h o -> p (a b) h o"),
                VB[:, 0:12, :, DH:DH + 1],
            )

            # ---- projections: fp8 DoubleRow 3-term chains (generators
            # yielding after each matmul so fillers can interleave) ----
            def qk_chain_g(pp_slice, w_hi, w_lo, hp, qc, bias_off):
                for j in range(KCP):
                    for term, (w_sb, x_sb) in enumerate(
                        ((w_hi, XH), (w_hi, XL), (w_lo, XH))
                    ):
                        nc.tensor.matmul(
                            pp_slice,
                            w_sb[:, j, :, hp * 128:(hp + 1) * 128],
                            x_sb[:, j, :, qc, :],
                            start=(j == 0 and term == 0),
                            stop=(j == KCP - 1 and term == 2 and not with_bias),
                            perf_mode=DR,
                        )
                        yield
                if with_bias:
                    nc.tensor.matmul(
                        pp_slice,
                        BQKV[0:1, bias_off + hp * 128:bias_off + (hp + 1) * 128],
                        ONES[0:1, qc * QW:(qc + 1) * QW],
                        start=False, stop=True,
                    )
                    yield

            def v_chain_g(pp_slice, st):
                qb, s0 = st // 4, (st % 4) * 128
                for j in range(KCP):
                    for term, (x_sb, w_sb) in enumerate(
                        ((XH, WVh), (XL, WVh), (XH, WVl))
                    ):
                        nc.tensor.matmul(
                            pp_slice,
                            x_sb[:, j, :, qb, s0:s0 + 128],
                            w_sb[:, j, :, :],
                            start=(j == 0 and term == 0),
                            stop=(j == KCP - 1 and term == 2 and not with_bias),
                            perf_mode=DR,
                        )
                        yield
                if with_bias:
                    nc.tensor.matmul(
                        pp_slice,
                        ONES[0:1, st * 128:(st + 1) * 128],
                        BQKV[0:1, 512:768],
                        start=False, stop=True,
                    )
                    yield

            class Filler:
                """FIFO of PE micro-work generators; m-loops pull a few
                matmuls at a time so proj/O-proj work interleaves with the
                ACT-paced attention stream."""

                def __init__(self):
                    self.units = []
                    self.cur = None
                    self.cur_marker = None

                def add(self, marker, genfn):
                    self.units.append((marker, genfn))

                def pull(self, n):
                    while n > 0:
                        if self.cur is None:
                            if not self.units:
                                return
                            self.cur_marker, genfn = self.units.pop(0)
                            self.cur = genfn()
                        try:
                            next(self.cur)
                            n -= 1
                        except StopIteration:
                            self.cur = None

                def drain_marker(self, marker):
                    while (self.cur is not None and self.cur_marker == marker) \
                            or any(m == marker for m, _ in self.units):
                        self.pull(1)

            # ---- projection chain-tile emitters (PE filler units) ----
            def qk_single_g(qc, hp, which):
                # Q/K chain as a psF single, generator form
                w_hi, w_lo, dst, boff = (
                    (WQh, WQl, QT, 0) if which == "q" else (WKh, WKl, KT, 256))
                pz = psF.tile([128, QW], F32, tag="f",
                              name=f"f{which}{qc}_{hp}")
                yield from qk_chain_g(pz[:], w_hi, w_lo, hp, qc, boff)
                if which == "q":
                    nc.vector.tensor_copy(dst[hp][:, qc, 0, :], pz[:])
                else:
                    nc.vector.tensor_copy(
                        dst[hp][:, 4 * qc:4 * qc + 4, 0, :],
                        pz[:].rearrange("p (k c) -> p k c", k=4))

            def v_single_g(st):
                pz = psF.tile([128, QW], F32, tag="f", name=f"fv{st}")
                yield from v_chain_g(pz[:, 0:256], st)
                nc.vector.tensor_copy(
                    VB[:, st, :, 0:DH],
                    pz[:, 0:256].rearrange("p (h d) -> p h d", h=HLOC),
                )
                if st < 12:
                    nc.vector.tensor_copy(
                        V8[:, st // 2, st % 2, :, 0:DH],
                        pz[:, 0:256].rearrange("p (h d) -> p h d", h=HLOC),
                    )

            def emit_qk_pair(qc, which):
                w_hi, w_lo, dst, boff = (
                    (WQh, WQl, QT, 0) if which == "q" else (WKh, WKl, KT, 256))
                pp = psS.tile([128, 2, QW], F32, tag="s", name=f"{which}p{qc}")
                for hp in range(2):
                    for _ in qk_chain_g(pp[:, hp, :], w_hi, w_lo, hp, qc, boff):
                        pass
                for hp in range(2):
                    if which == "q":
                        nc.vector.tensor_copy(dst[hp][:, qc, 0, :], pp[:, hp, :])
                    else:
                        nc.vector.tensor_copy(
                            dst[hp][:, 4 * qc:4 * qc + 4, 0, :],
                            pp[:, hp, :].rearrange("p (k c) -> p k c", k=4))

            def emit_v_pair(st2):
                pp = psS.tile([128, 2, QW], F32, tag="s", name=f"vp{st2}")
                for i in range(2):
                    for _ in v_chain_g(pp[:, i, 0:256], st2 * 2 + i):
                        pass
                for i in range(2):
                    st = st2 * 2 + i
                    nc.vector.tensor_copy(
                        VB[:, st, :, 0:DH],
                        pp[:, i, 0:256].rearrange("p (h d) -> p h d", h=HLOC),
                    )
                    if st < 12:
                        nc.vector.tensor_copy(
                            V8[:, st // 2, st % 2, :, 0:DH],
                            pp[:, i, 0:256].rearrange("p (h d) -> p h d", h=HLOC),
                        )

            # ---- output projection units (PE filler; one (qt,mc) each) ----
            ZTS = {}   # qc -> [ZT_hp0, ZT_hp1]
            OSB = {}   # (qc, qt) -> staging tile; DMA fires after mc=1

            def o_single_g(qc, qt, mc):
                q0 = qc * QW
                ZT = ZTS[qc]
                t = psF.tile([128, QW], F32, tag="f", name=f"o{qc}_{qt}_{mc}")
                if mc == 0:
                    OSB[(qc, qt)] = pout.tile([128, DM], F32, tag="ob",
                                              name=f"ob{qc}_{qt}")
                osb = OSB[(qc, qt)]
                for hp in range(2):
                    nc.tensor.matmul(
                        t[:],
                        ZT[hp][:, qt * 128:(qt + 1) * 128],
                        WO[:, hp, mc * QW:(mc + 1) * QW],
                        start=(hp == 0), stop=(hp == 1),
                    )
                    yield
                if qc == 3 and (qt + mc) % 2 == 1:
                    # tail: ACT is idle after the last exp; splitting the
                    # final copies across DVE+ACT halves the drain
                    nc.scalar.copy(osb[:, mc * QW:(mc + 1) * QW], t[:])
                else:
                    nc.vector.tensor_copy(osb[:, mc * QW:(mc + 1) * QW], t[:])
                if mc == 1:
                    nc.sync.dma_start(
                        out[q0 + qt * 128:q0 + (qt + 1) * 128, :], osb[:])

            def emit_o_unit(qc, qt):
                for mc in range(2):
                    for _ in o_single_g(qc, qt, mc):
                        pass

            # ---- one attention head: scores->exp->Z with the Z matmuls
            # delayed one s-tile behind (exp latency hiding) and filler
            # matmuls pulled between iterations ----
            norm_pend = []   # deferred (bcast-mm + ZT-mul) closures

            def drain_norm():
                while norm_pend:
                    norm_pend.pop(0)()

            def emit_head(qc, h, filler):
                hp, hh = h // 2, h % 2
                hs = slice(hh * DH, (hh + 1) * DH)
                zps = psZ.tile([128, QW], F32, tag="z", name=f"z{qc}_{h}")

                def s_mm(dst, kt, q_lo, q_hi):
                    nc.tensor.matmul(
                        dst,
                        KT[hp][hs, kt, :, :],
                        QT[hp][hs, qc, :, q_lo:q_hi],
                        start=True, stop=True,
                        perf_mode=DR,
                    )

                def z_dr(m, pt8):
                    nc.tensor.matmul(
                        zps[0:DH + 1, :], V8[:, m, :, h, 0:DH + 1], pt8[:],
                        start=(m == 0), stop=False,
                        perf_mode=DR, skip_group_check=True,
                    )

                pend = []
                for m in range(2 * qc):
                    sps = psS.tile([128, 2, QW], F32, tag="s",
                                   name=f"s{qc}_{h}_{m}")
                    for i in range(2):
                        s_mm(sps[:, i, :], 2 * m + i, 0, QW)
                    drain_norm()  # deferred prev-head normalize on PE slack
                    pt8 = ppt8.tile([128, 2, QW], F8, tag="p8",
                                    name=f"p8_{qc}_{h}_{m}")
                    nc.scalar.activation(
                        pt8[:], sps[:], mybir.ActivationFunctionType.Exp,
                        scale=EXP_SCALE,
                    )
                    pend.append((z_dr, m, pt8))
                    filler.pull(3)
                    if len(pend) > 1:
                        fn, mm, pt = pend.pop(0)
                        fn(mm, pt)

                ktA, ktB = 4 * qc, 4 * qc + 2
                sA = psS.tile([128, 2, QW], F32, tag="s", name=f"sA{qc}_{h}")
                s_mm(sA[:, 0, :], ktA, 0, QW)
                s_mm(sA[:, 1, :], ktA + 1, 0, QW)
                drain_norm()
                ptA = pptb.tile([128, 2, QW], BF16, tag="pb", name=f"pA{qc}_{h}")
                nc.scalar.activation(
                    ptA[:], sA[:], mybir.ActivationFunctionType.Exp,
                    scale=EXP_SCALE,
                )
                for fn, mm, pt in pend:
                    fn(mm, pt)
                sB = psS.tile([128, 2, QW], F32, tag="s", name=f"sB{qc}_{h}")
                s_mm(sB[:, 0, 0:256], ktB, 256, QW)
                s_mm(sB[:, 0, 256:QW], ktB + 1, 256, QW)
                ptB = pptb.tile([128, 2, QW], BF16, tag="pb", name=f"pB{qc}_{h}")
                nc.scalar.activation(
                    ptB[:, 0, :], sB[:, 0, :],
                    mybir.ActivationFunctionType.Exp, scale=EXP_SCALE,
                )
                nc.gpsimd.tensor_mul(ptA[:, 0, 0:128], ptA[:, 0, 0:128], TRI[:])
                nc.gpsimd.tensor_mul(ptA[:, 1, 128:256], ptA[:, 1, 128:256], TRI[:])
                nc.gpsimd.tensor_mul(ptB[:, 0, 0:128], ptB[:, 0, 0:128], TRI[:])
                nc.gpsimd.tensor_mul(ptB[:, 0, 384:QW], ptB[:, 0, 384:QW], TRI[:])

                # PE filler while the diagonal exps+masks complete
                filler.pull(4)

                def z_bf(v_st, pt_ap, q_lo, q_hi, start, stop):
                    nc.tensor.matmul(
                        zps[0:DH + 1, q_lo:q_hi],
                        VB[:, v_st, h, :], pt_ap,
                        start=start, stop=stop, skip_group_check=True,
                    )

                z_bf(ktA, ptA[:, 0, :], 0, QW, qc == 0, False)
                z_bf(ktA + 1, ptA[:, 1, 128:QW], 128, QW, False, False)
                z_bf(ktB, ptB[:, 0, 0:256], 256, QW, False, False)
                z_bf(ktB + 1, ptB[:, 0, 384:QW], 384, QW, False, True)

                recip = prs.tile([1, QW], F32R, tag="recip", name=f"rc{qc}_{h}")
                with nc.allow_low_precision(reason="softmax recip in fp32r"):
                    nc.vector.reciprocal(recip[:], zps[DH:DH + 1, :])
                rb = prs.tile([DH, QW], F32R, tag="rb", name=f"rb{qc}_{h}")
                nc.gpsimd.partition_broadcast(rb[:], recip[:])
                if qc not in ZTS:
                    ZTS[qc] = [
                        pzt.tile([128, QW], BF16, tag="zt", name=f"zt{qc}_{p}")
                        for p in range(2)
                    ]
                nc.vector.tensor_mul(ZTS[qc][hp][hs, :], zps[0:DH, :], rb[:])

            # ---- global pipelined emission ----
            # minimal prefix so the first exp fires early: Q-hp0, K pair,
            # V pairs for qc0 diag; everything else rides the filler queue
            for _ in qk_single_g(0, 0, "q"):
                pass
            emit_qk_pair(0, "k")
            for st in range(4):
                for _ in v_single_g(st):
                    pass

            filler = Filler()
            filler.add("p0", lambda: qk_single_g(0, 1, "q"))
            for qcn in (1, 2, 3):
                for which in ("q", "k"):
                    for hp in range(2):
                        filler.add(f"p{qcn}",
                                   (lambda qcn=qcn, hp=hp, which=which:
                                    qk_single_g(qcn, hp, which)))
                for st in range(4 * qcn, 4 * qcn + 4):
                    filler.add(f"p{qcn}", lambda st=st: v_single_g(st))
            def add_o_units(qcp):
                # added only after head(qcp,3) is emitted so no O matmul can
                # be pulled ahead of its ZT writes
                for qt in range(4):
                    for mc in range(2):
                        filler.add(f"o{qcp}",
                                   (lambda qcp=qcp, qt=qt, mc=mc:
                                    o_single_g(qcp, qt, mc)))

            # head order: qc-major, but (3,0) hoisted before (2,3) so qc3's
            # ACT work starts earlier
            order = [(0, 0), (0, 1), (0, 2), (0, 3),
                     (1, 0), (1, 1), (1, 2), (1, 3),
                     (2, 0), (2, 1), (2, 2), (3, 0), (2, 3),
                     (3, 1), (3, 2), (3, 3)]
            for qc, h in order:
                if h == 0:
                    filler.drain_marker(f"p{qc}")
                if qc == 0 and h == 2:
                    filler.drain_marker("p0")  # QT hp1 needed
                emit_head(qc, h, filler)
                if h == 3 and qc < 3:
                    add_o_units(qc)
            drain_norm()
            filler.drain_marker("o2")
            for qt in range(4):
                emit_o_unit(3, qt)

    nc.compile()
    return nc


def _get_program(with_bias: bool):
    if with_bias not in _PROGRAMS:
        _PROGRAMS[with_bias] = _build(with_bias)
    return _PROGRAMS[with_bias]


def _split8(a):
    hi = a.astype(ml_dtypes.float8_e4m3)
    lo = (a - hi.astype(np.float32)).astype(ml_dtypes.float8_e4m3)
    return hi, lo


def _x_layout(a):
    # [1024, 2048] -> [128, j, i, qb, s]
    return np.ascontiguousarray(
        a.reshape(KCP, 2, 128, NQC, QW).transpose(2, 0, 1, 3, 4))


def _w_layout(a):
    # [1024, 256] -> [128, j, i, 256]
    return np.ascontiguousarray(
        a.reshape(KCP, 2, 128, 256).transpose(2, 0, 1, 3))


def kernel(normalized_resid_pre, W_Q, W_K, W_V, W_O, b_Q, b_K, b_V, b_O):
    x = np.asarray(normalized_resid_pre, dtype=np.float32)
    W_Q = np.asarray(W_Q, dtype=np.float32)
    W_K = np.asarray(W_K, dtype=np.float32)
    W_V = np.asarray(W_V, dtype=np.float32)
    W_O = np.asarray(W_O, dtype=np.float32)
    b_Q = np.asarray(b_Q, dtype=np.float32)
    b_K = np.asarray(b_K, dtype=np.float32)
    b_V = np.asarray(b_V, dtype=np.float32)
    b_O = np.asarray(b_O, dtype=np.float32)

    batch, seq, dm = x.shape
    with_bias = bool(np.any(b_Q) or np.any(b_K) or np.any(b_V))
    nc = _get_program(with_bias)

    tri = np.ascontiguousarray(
        np.triu(np.ones((128, 128), np.float32)).astype(ml_dtypes.bfloat16))

    xsp = []
    for b in range(batch):
        xh, xl = _split8(np.ascontiguousarray(x[b].T))
        xsp.append((_x_layout(xh), _x_layout(xl)))

    in_maps = []
    for c in range(8):
        b, g = c // 4, c % 4
        hs = slice(4 * g, 4 * g + 4)
        m = {"xh8": xsp[b][0], "xl8": xsp[b][1], "tri": tri,
             "z8": np.zeros((128, 2048), ml_dtypes.float8_e4m3)}
        for nm, W in (("wq", W_Q), ("wk", W_K), ("wv", W_V)):
            Wp = np.transpose(W[hs], (1, 0, 2)).reshape(dm, 256) * WS
            hi, lo = _split8(Wp)
            m[nm + "h"] = _w_layout(hi)
            m[nm + "l"] = _w_layout(lo)
        m["wo"] = np.ascontiguousarray(
            (W_O[hs].reshape(256, dm) / WS).astype(ml_dtypes.bfloat16))
        if with_bias:
            m["bqkv"] = np.ascontiguousarray(np.concatenate(
                [b_Q[hs].reshape(256) * WS, b_K[hs].reshape(256) * WS,
                 b_V[hs].reshape(256) * WS]
            )[None, :].astype(ml_dtypes.bfloat16))
            m["ones"] = np.ones((1, seq), ml_dtypes.bfloat16)
        in_maps.append(m)

    res = bass_utils.run_bass_kernel_spmd(nc, in_maps, core_ids=list(range(8)))
    parts = [res.results[c]["out"] for c in range(8)]
    full = np.stack(
        [parts[0] + parts[1] + parts[2] + parts[3],
         parts[4] + parts[5] + parts[6] + parts[7]]
    )
    full += b_O
    return full.astype(np.float32)


# revision 53
# speedup vs baseline: 1.0205x; 1.0076x over previous
"""Multi-head causal attention (B=2, S=2048, D=1024, H=16, Dh=64) on 8 TRN2 cores.

Sharding: tensor-parallel over 4 head-groups x data-parallel over 2 batches.
Core c handles batch c//4, heads [4*(c%4), 4*(c%4)+4). Each core computes its
partial output projection; the host sums the 4 partials per batch (the
"all-reduce") and adds b_O.

Mixed-precision device program (fp32 PSUM accumulation everywhere):
  - Q/K/V projections: fp8e4m3 DoubleRow matmuls (0.5 cyc/row, 256-deep
    contraction per instr). Host pre-splits x and 64*W into hi/lo fp8 pairs;
    3 compensation terms xh@Wh + xl@Wh + xh@Wl recover ~bf16 accuracy.
  - Scores S = (64q)(64k): bf16 QT/KT, per-128-column causal crops. The /8
    softmax scale and the 1/4096 fp8 weight scaling fold into the ACT exp
    (scale=1/32768).
  - P = exp(S): ACT writes fp8 tiles for off-diagonal k-pairs, bf16 for the
    4 diagonal k-tiles (where P concentrates and fp8 noise would not average
    out). Triangular masks multiply bf16 diag tiles only (DVE 2x on bf16).
  - Z = P @ [64V|1]: off-diag via fp8 DoubleRow over k-tile pairs (0.25
    cyc/row/k-tile), diag via bf16-moving matmuls; row 64 accumulates the
    softmax denominator.
  - out += (Z/denom)^T @ (Wo/64): f32r stationary ZT, bf16 moving WO.
"""

import numpy as np
import ml_dtypes

import concourse.mybir as mybir
import concourse.tile as tile
from concourse import bacc
from concourse import bass_utils

F32 = mybir.dt.float32
F32R = mybir.dt.float32r
BF16 = mybir.dt.bfloat16
F8 = mybir.dt.float8e4

SEQ = 2048
DM = 1024
DH = 64
HLOC = 4          # heads per core
KCP = 4           # dmodel pair-chunks of 256
NQC = 4           # q chunks of 512
QW = 512
WS = 64.0         # fp8 weight scale
EXP_SCALE = 1.0 / (WS * WS * 8.0)
DR = mybir.MatmulPerfMode.DoubleRow

_PROGRAMS = {}


def _build(with_bias: bool):
    nc = bacc.Bacc("TRN2", target_bir_lowering=False, debug=False, num_devices=8)

    # [128, j(4), i(2), qb(4), 512] with element [p,j,i,qb,s] = xT[256j+128i+p,
    # 512qb+s]; hi/lo fp8 split of xT
    xh8 = nc.dram_tensor("xh8", [128, KCP, 2, NQC, QW], F8, kind="ExternalInput").ap()
    xl8 = nc.dram_tensor("xl8", [128, KCP, 2, NQC, QW], F8, kind="ExternalInput").ap()
    # [128, j(4), i(2), 256]: hi/lo of 64*W[256j+128i+p, m]
    wname = lambda n: nc.dram_tensor(n, [128, KCP, 2, 256], F8, kind="ExternalInput").ap()
    wqh, wql = wname("wqh"), wname("wql")
    wkh, wkl = wname("wkh"), wname("wkl")
    wvh, wvl = wname("wvh"), wname("wvl")
    wo = nc.dram_tensor("wo", [256, DM], BF16, kind="ExternalInput").ap()
    z8 = nc.dram_tensor("z8", [128, 2048], F8, kind="ExternalInput").ap()
    tri = nc.dram_tensor("tri", [128, 128], BF16, kind="ExternalInput").ap()
    if with_bias:
        bqkv = nc.dram_tensor("bqkv", [1, 768], BF16, kind="ExternalInput").ap()
        ones = nc.dram_tensor("ones", [1, SEQ], BF16, kind="ExternalInput").ap()
    out = nc.dram_tensor("out", [SEQ, DM], F32, kind="ExternalOutput").ap()

    with tile.TileContext(nc) as tc:
        with (
            tc.tile_pool(name="px", bufs=1) as px,
            tc.tile_pool(name="pw", bufs=1) as pw,
            tc.tile_pool(name="pqk", bufs=1) as pqk,
            tc.tile_pool(name="pv", bufs=1) as pv,
            tc.tile_pool(name="ppt8", bufs=4) as ppt8,
            tc.tile_pool(name="pptb", bufs=4) as pptb,
            tc.tile_pool(name="pzt", bufs=4) as pzt,
            tc.tile_pool(name="prs", bufs=3) as prs,
            tc.tile_pool(name="pout", bufs=4) as pout,
            tc.tile_pool(name="psS", bufs=2, space="PSUM") as psS,
            tc.tile_pool(name="psZ", bufs=2, space="PSUM") as psZ,
            tc.tile_pool(name="psF", bufs=2, space="PSUM") as psF,
        ):
            # ---- SBUF tiles ----
            XH = px.tile([128, KCP, 2, NQC, QW], F8, tag="xh")
            XL = px.tile([128, KCP, 2, NQC, QW], F8, tag="xl")
            WQh = pw.tile([128, KCP, 2, 256], F8, tag="wqh")
            WQl = pw.tile([128, KCP, 2, 256], F8, tag="wql")
            WKh = pw.tile([128, KCP, 2, 256], F8, tag="wkh")
            WKl = pw.tile([128, KCP, 2, 256], F8, tag="wkl")
            WVh = pw.tile([128, KCP, 2, 256], F8, tag="wvh")
            WVl = pw.tile([128, KCP, 2, 256], F8, tag="wvl")

            QT = [pqk.tile([128, NQC, 2, QW], F8, tag=f"qt{hp}", name=f"qt{hp}")
                  for hp in range(2)]
            KT = [pqk.tile([128, 16, 2, 128], F8, tag=f"kt{hp}", name=f"kt{hp}")
                  for hp in range(2)]

            # ---- input DMA: qb0 x-blocks + QK weights first ----
            nc.sync.dma_start(WQh[:], wqh)
            nc.sync.dma_start(WQl[:], wql)
            nc.sync.dma_start(XH[:, 0, :, 0, :], xh8[:, 0, :, 0, :])
            nc.sync.dma_start(XL[:, 0, :, 0, :], xl8[:, 0, :, 0, :])
            nc.sync.dma_start(XH[:, 1:4, :, 0, :], xh8[:, 1:4, :, 0, :])
            nc.sync.dma_start(XL[:, 1:4, :, 0, :], xl8[:, 1:4, :, 0, :])
            nc.sync.dma_start(WKh[:], wkh)
            nc.sync.dma_start(WKl[:], wkl)
            for hp in range(2):
                nc.sync.dma_start(
                    QT[hp][:, :, 1, :],
                    z8[:, 0:NQC * QW].rearrange("p (a b) -> p a b", a=NQC))
                nc.sync.dma_start(
                    KT[hp][:, :, 1, :],
                    z8[:, 0:2048].rearrange("p (a b) -> p a b", a=16))
            nc.sync.dma_start(WVh[:], wvh)
            nc.sync.dma_start(WVl[:], wvl)
            TRI = pw.tile([128, 128], BF16, tag="tri")
            nc.sync.dma_start(TRI[:], tri)
            nc.sync.dma_start(XH[:, :, :, 1, :], xh8[:, :, :, 1, :])
            nc.sync.dma_start(XL[:, :, :, 1, :], xl8[:, :, :, 1, :])
            WO = pw.tile([128, 2, DM], BF16, tag="wo")
            for hp in range(2):
                nc.sync.dma_start(WO[:, hp, :], wo[hp * 128:(hp + 1) * 128, :])
            for qb in range(2, NQC):
                nc.sync.dma_start(XH[:, :, :, qb, :], xh8[:, :, :, qb, :])
                nc.sync.dma_start(XL[:, :, :, qb, :], xl8[:, :, :, qb, :])
            if with_bias:
                BQKV = pw.tile([1, 768], BF16, tag="bqkv")
                nc.sync.dma_start(BQKV[:], bqkv)
                ONES = pw.tile([1, SEQ], BF16, tag="ones")
                nc.sync.dma_start(ONES[:], ones)

            # QT/KT fp8 with an interleaved zero slot: scores run as
            # zero-padded DoubleRow matmuls at 0.5 cyc/row. Slot 1 of both
            # stationary and moving operands is zeroed via DMA so the second
            # dual-row tile contributes exactly 0.
            VB = pv.tile([128, 16, HLOC, DH + 1], BF16, tag="vb")
            # innermost padded to 68 so the DoubleRow pair stride (4*68)
            # is 16-element aligned (s3_lw_dual_fp8 ISA restriction)
            V8 = pv.tile([128, 6, 2, HLOC, DH + 4], F8, tag="v8")
            nc.vector.memset(VB[:, :, :, DH:DH + 1], 1.0)
            nc.vector.tensor_copy(
                V8[:, :, :, :, DH:DH + 1].rearrange("p a b h o -> p (a b) h o"),
                VB[:, 0:12, :, DH:DH + 1],
            )

            # ---- projections: fp8 DoubleRow 3-term chains (generators
            # yielding after each matmul so fillers can interleave) ----
            def qk_chain_g(pp_slice, w_hi, w_lo, hp, qc, bias_off):
                for j in range(KCP):
                    for term, (w_sb, x_sb) in enumerate(
                        ((w_hi, XH), (w_hi, XL), (w_lo, XH))
                    ):
                        nc.tensor.matmul(
                            pp_slice,
                            w_sb[:, j, :, hp * 128:(hp + 1) * 128],
                            x_sb[:, j, :, qc, :],
                            start=(j == 0 and term == 0),
                            stop=(j == KCP - 1 and term == 2 and not with_bias),
                            perf_mode=DR,
                        )
                        yield
                if with_bias:
                    nc.tensor.matmul(
                        pp_slice,
                        BQKV[0:1, bias_off + hp * 128:bias_off + (hp + 1) * 128],
                        ONES[0:1, qc * QW:(qc + 1) * QW],
                        start=False, stop=True,
                    )
                    yield

            def v_chain_g(pp_slice, st):
                qb, s0 = st // 4, (st % 4) * 128
                for j in range(KCP):
                    for term, (x_sb, w_sb) in enumerate(
                        ((XH, WVh), (XL, WVh), (XH, WVl))
                    ):
                        nc.tensor.matmul(
                            pp_slice,
                            x_sb[:, j, :, qb, s0:s0 + 128],
                            w_sb[:, j, :, :],
                            start=(j == 0 and term == 0),
                            stop=(j == KCP - 1 and term == 2 and not with_bias),
                            perf_mode=DR,
                        )
                        yield
                if with_bias:
                    nc.tensor.matmul(
                        pp_slice,
                        ONES[0:1, st * 128:(st + 1) * 128],
                        BQKV[0:1, 512:768],
                        start=False, stop=True,
                    )
                    yield

            class Filler:
                """FIFO of PE micro-work generators; m-loops pull a few
                matmuls at a time so proj/O-proj work interleaves with the
                ACT-paced attention stream."""

                def __init__(self):
                    self.units = []
                    self.cur = None
                    self.cur_marker = None

                def add(self, marker, genfn):
                    self.units.append((marker, genfn))

                def pull(self, n):
                    while n > 0:
                        if self.cur is None:
                            if not self.units:
                                return
                            self.cur_marker, genfn = self.units.pop(0)
                            self.cur = genfn()
                        try:
                            next(self.cur)
                            n -= 1
                        except StopIteration:
                            self.cur = None

                def drain_marker(self, marker):
                    while (self.cur is not None and self.cur_marker == marker) \
                            or any(m == marker for m, _ in self.units):
                        self.pull(1)

            # ---- projection chain-tile emitters (PE filler units) ----
            def qk_single_g(qc, hp, which):
                # Q/K chain as a psF single, generator form
                w_hi, w_lo, dst, boff = (
                    (WQh, WQl, QT, 0) if which == "q" else (WKh, WKl, KT, 256))
                pz = psF.tile([128, QW], F32, tag="f",
                              name=f"f{which}{qc}_{hp}")
                yield from qk_chain_g(pz[:], w_hi, w_lo, hp, qc, boff)
                eng = nc.scalar if qc <= 1 else nc.vector
                if which == "q":
                    eng.copy(dst[hp][:, qc, 0, :], pz[:]) if qc <= 1 else \
                        nc.vector.tensor_copy(dst[hp][:, qc, 0, :], pz[:])
                else:
                    if qc <= 1:
                        eng.copy(dst[hp][:, 4 * qc:4 * qc + 4, 0, :],
                                 pz[:].rearrange("p (k c) -> p k c", k=4))
                    else:
                        nc.vector.tensor_copy(
                            dst[hp][:, 4 * qc:4 * qc + 4, 0, :],
                            pz[:].rearrange("p (k c) -> p k c", k=4))

            def v_single_g(st):
                pz = psF.tile([128, QW], F32, tag="f", name=f"fv{st}")
                yield from v_chain_g(pz[:, 0:256], st)
                nc.vector.tensor_copy(
                    VB[:, st, :, 0:DH],
                    pz[:, 0:256].rearrange("p (h d) -> p h d", h=HLOC),
                )
                if st < 12:
                    nc.vector.tensor_copy(
                        V8[:, st // 2, st % 2, :, 0:DH],
                        pz[:, 0:256].rearrange("p (h d) -> p h d", h=HLOC),
                    )

            def emit_qk_pair(qc, which):
                w_hi, w_lo, dst, boff = (
                    (WQh, WQl, QT, 0) if which == "q" else (WKh, WKl, KT, 256))
                pp = psS.tile([128, 2, QW], F32, tag="s", name=f"{which}p{qc}")
                for hp in range(2):
                    for _ in qk_chain_g(pp[:, hp, :], w_hi, w_lo, hp, qc, boff):
                        pass
                for hp in range(2):
                    if which == "q":
                        nc.scalar.copy(dst[hp][:, qc, 0, :], pp[:, hp, :])
                    else:
                        nc.scalar.copy(
                            dst[hp][:, 4 * qc:4 * qc + 4, 0, :],
                            pp[:, hp, :].rearrange("p (k c) -> p k c", k=4))

            def emit_v_pair(st2):
                pp = psS.tile([128, 2, QW], F32, tag="s", name=f"vp{st2}")
                for i in range(2):
                    for _ in v_chain_g(pp[:, i, 0:256], st2 * 2 + i):
                        pass
                for i in range(2):
                    st = st2 * 2 + i
                    nc.vector.tensor_copy(
                        VB[:, st, :, 0:DH],
                        pp[:, i, 0:256].rearrange("p (h d) -> p h d", h=HLOC),
                    )
                    if st < 12:
                        nc.vector.tensor_copy(
                            V8[:, st // 2, st % 2, :, 0:DH],
                            pp[:, i, 0:256].rearrange("p (h d) -> p h d", h=HLOC),
                        )

            # ---- output projection units (PE filler; one (qt,mc) each) ----
            ZTS = {}   # qc -> [ZT_hp0, ZT_hp1]
            OSB = {}   # (qc, qt) -> staging tile; DMA fires after mc=1

            def o_single_g(qc, qt, mc):
                q0 = qc * QW
                ZT = ZTS[qc]
                t = psF.tile([128, QW], F32, tag="f", name=f"o{qc}_{qt}_{mc}")
                if mc == 0:
                    OSB[(qc, qt)] = pout.tile([128, DM], F32, tag="ob",
                                              name=f"ob{qc}_{qt}")
                osb = OSB[(qc, qt)]
                for hp in range(2):
                    nc.tensor.matmul(
                        t[:],
                        ZT[hp][:, qt * 128:(qt + 1) * 128],
                        WO[:, hp, mc * QW:(mc + 1) * QW],
                        start=(hp == 0), stop=(hp == 1),
                    )
                    yield
                if qc == 3 and (qt + mc) % 2 == 1:
                    # tail: ACT is idle after the last exp; splitting the
                    # final copies across DVE+ACT halves the drain
                    nc.scalar.copy(osb[:, mc * QW:(mc + 1) * QW], t[:])
                else:
                    nc.vector.tensor_copy(osb[:, mc * QW:(mc + 1) * QW], t[:])
                if qc == 3:
                    nc.sync.dma_start(
                        out[q0 + qt * 128:q0 + (qt + 1) * 128,
                            mc * QW:(mc + 1) * QW],
                        osb[:, mc * QW:(mc + 1) * QW])
                elif mc == 1:
                    nc.sync.dma_start(
                        out[q0 + qt * 128:q0 + (qt + 1) * 128, :], osb[:])

            def emit_o_unit(qc, qt):
                for mc in range(2):
                    for _ in o_single_g(qc, qt, mc):
                        pass

            # ---- one attention head: scores->exp->Z with the Z matmuls
            # delayed one s-tile behind (exp latency hiding) and filler
            # matmuls pulled between iterations ----
            norm_pend = []   # deferred (bcast-mm + ZT-mul) closures

            def drain_norm():
                while norm_pend:
                    norm_pend.pop(0)()

            def emit_head(qc, h, filler):
                hp, hh = h // 2, h % 2
                hs = slice(hh * DH, (hh + 1) * DH)
                zps = psZ.tile([128, QW], F32, tag="z", name=f"z{qc}_{h}")

                def s_mm(dst, kt, q_lo, q_hi):
                    nc.tensor.matmul(
                        dst,
                        KT[hp][hs, kt, :, :],
                        QT[hp][hs, qc, :, q_lo:q_hi],
                        start=True, stop=True,
                        perf_mode=DR,
                    )

                def z_dr(m, pt8):
                    nc.tensor.matmul(
                        zps[0:DH + 1, :], V8[:, m, :, h, 0:DH + 1], pt8[:],
                        start=(m == 0), stop=False,
                        perf_mode=DR, skip_group_check=True,
                    )

                pend = []
                for m in range(2 * qc):
                    sps = psS.tile([128, 2, QW], F32, tag="s",
                                   name=f"s{qc}_{h}_{m}")
                    for i in range(2):
                        s_mm(sps[:, i, :], 2 * m + i, 0, QW)
                    drain_norm()  # deferred prev-head normalize on PE slack
                    pt8 = ppt8.tile([128, 2, QW], F8, tag="p8",
                                    name=f"p8_{qc}_{h}_{m}")
                    nc.scalar.activation(
                        pt8[:], sps[:], mybir.ActivationFunctionType.Exp,
                        scale=EXP_SCALE,
                    )
                    pend.append((z_dr, m, pt8))
                    filler.pull(3)
                    if len(pend) > 1:
                        fn, mm, pt = pend.pop(0)
                        fn(mm, pt)

                ktA, ktB = 4 * qc, 4 * qc + 2
                sA = psS.tile([128, 2, QW], F32, tag="s", name=f"sA{qc}_{h}")
                s_mm(sA[:, 0, :], ktA, 0, QW)
                s_mm(sA[:, 1, :], ktA + 1, 0, QW)
                drain_norm()
                ptA = pptb.tile([128, 2, QW], BF16, tag="pb", name=f"pA{qc}_{h}")
                nc.scalar.activation(
                    ptA[:], sA[:], mybir.ActivationFunctionType.Exp,
                    scale=EXP_SCALE,
                )
                for fn, mm, pt in pend:
                    fn(mm, pt)
                sB = psS.tile([128, 2, QW], F32, tag="s", name=f"sB{qc}_{h}")
                s_mm(sB[:, 0, 0:256], ktB, 256, QW)
                s_mm(sB[:, 0, 256:QW], ktB + 1, 256, QW)
                ptB = pptb.tile([128, 2, QW], BF16, tag="pb", name=f"pB{qc}_{h}")
                nc.scalar.activation(
                    ptB[:, 0, :], sB[:, 0, :],
                    mybir.ActivationFunctionType.Exp, scale=EXP_SCALE,
                )
                meng = nc.vector if (qc == 3 and h == 3) else nc.gpsimd
                meng.tensor_mul(ptA[:, 0, 0:128], ptA[:, 0, 0:128], TRI[:])
                meng.tensor_mul(ptA[:, 1, 128:256], ptA[:, 1, 128:256], TRI[:])
                meng.tensor_mul(ptB[:, 0, 0:128], ptB[:, 0, 0:128], TRI[:])
                meng.tensor_mul(ptB[:, 0, 384:QW], ptB[:, 0, 384:QW], TRI[:])

                # PE filler while the diagonal exps+masks complete
                filler.pull(4)

                def z_bf(v_st, pt_ap, q_lo, q_hi, start, stop):
                    nc.tensor.matmul(
                        zps[0:DH + 1, q_lo:q_hi],
                        VB[:, v_st, h, :], pt_ap,
                        start=start, stop=stop, skip_group_check=True,
                    )

                z_bf(ktA, ptA[:, 0, :], 0, QW, qc == 0, False)
                z_bf(ktA + 1, ptA[:, 1, 128:QW], 128, QW, False, False)
                z_bf(ktB, ptB[:, 0, 0:256], 256, QW, False, False)
                z_bf(ktB + 1, ptB[:, 0, 384:QW], 384, QW, False, True)

                recip = prs.tile([1, QW], F32R, tag="recip", name=f"rc{qc}_{h}")
                with nc.allow_low_precision(reason="softmax recip in fp32r"):
                    nc.vector.reciprocal(recip[:], zps[DH:DH + 1, :])
                rb = prs.tile([DH, QW], F32R, tag="rb", name=f"rb{qc}_{h}")
                nc.gpsimd.partition_broadcast(rb[:], recip[:])
                if qc not in ZTS:
                    ZTS[qc] = [
                        pzt.tile([128, QW], BF16, tag="zt", name=f"zt{qc}_{p}")
                        for p in range(2)
                    ]
                nc.vector.tensor_mul(ZTS[qc][hp][hs, :], zps[0:DH, :], rb[:])

            # ---- global pipelined emission ----
            # minimal prefix so the first exp fires early: Q-hp0, K pair,
            # V pairs for qc0 diag; everything else rides the filler queue
            for _ in qk_single_g(0, 0, "q"):
                pass
            emit_qk_pair(0, "k")
            for st in range(4):
                for _ in v_single_g(st):
                    pass

            filler = Filler()
            filler.add("p0", lambda: qk_single_g(0, 1, "q"))
            for qcn in (1, 2, 3):
                for which in ("q", "k"):
                    for hp in range(2):
                        filler.add(f"p{qcn}",
                                   (lambda qcn=qcn, hp=hp, which=which:
                                    qk_single_g(qcn, hp, which)))
                for st in range(4 * qcn, 4 * qcn + 4):
                    filler.add(f"p{qcn}", lambda st=st: v_single_g(st))
            def add_o_units(qcp):
                # added only after head(qcp,3) is emitted so no O matmul can
                # be pulled ahead of its ZT writes
                for qt in range(4):
                    for mc in range(2):
                        filler.add(f"o{qcp}",
                                   (lambda qcp=qcp, qt=qt, mc=mc:
                                    o_single_g(qcp, qt, mc)))

            # head order: qc-major, but (3,0) hoisted before (2,3) so qc3's
            # ACT work starts earlier
            order = [(0, 0), (0, 1), (0, 2), (0, 3),
                     (1, 0), (1, 1), (1, 2), (1, 3),
                     (2, 0), (2, 1), (2, 2), (3, 0), (2, 3),
                     (3, 1), (3, 2), (3, 3)]
            for qc, h in order:
                if h == 0:
                    filler.drain_marker(f"p{qc}")
                if qc == 0 and h == 2:
                    filler.drain_marker("p0")  # QT hp1 needed
                emit_head(qc, h, filler)
                if h == 3 and qc < 3:
                    add_o_units(qc)
            drain_norm()
            filler.drain_marker("o2")
            for qt in range(4):
                emit_o_unit(3, qt)

    nc.compile()
    return nc


def _get_program(with_bias: bool):
    if with_bias not in _PROGRAMS:
        _PROGRAMS[with_bias] = _build(with_bias)
    return _PROGRAMS[with_bias]


def _split8(a):
    hi = a.astype(ml_dtypes.float8_e4m3)
    lo = (a - hi.astype(np.float32)).astype(ml_dtypes.float8_e4m3)
    return hi, lo


def _x_layout(a):
    # [1024, 2048] -> [128, j, i, qb, s]
    return np.ascontiguousarray(
        a.reshape(KCP, 2, 128, NQC, QW).transpose(2, 0, 1, 3, 4))


def _w_layout(a):
    # [1024, 256] -> [128, j, i, 256]
    return np.ascontiguousarray(
        a.reshape(KCP, 2, 128, 256).transpose(2, 0, 1, 3))


def kernel(normalized_resid_pre, W_Q, W_K, W_V, W_O, b_Q, b_K, b_V, b_O):
    x = np.asarray(normalized_resid_pre, dtype=np.float32)
    W_Q = np.asarray(W_Q, dtype=np.float32)
    W_K = np.asarray(W_K, dtype=np.float32)
    W_V = np.asarray(W_V, dtype=np.float32)
    W_O = np.asarray(W_O, dtype=np.float32)
    b_Q = np.asarray(b_Q, dtype=np.float32)
    b_K = np.asarray(b_K, dtype=np.float32)
    b_V = np.asarray(b_V, dtype=np.float32)
    b_O = np.asarray(b_O, dtype=np.float32)

    batch, seq, dm = x.shape
    with_bias = bool(np.any(b_Q) or np.any(b_K) or np.any(b_V))
    nc = _get_program(with_bias)

    tri = np.ascontiguousarray(
        np.triu(np.ones((128, 128), np.float32)).astype(ml_dtypes.bfloat16))

    xsp = []
    for b in range(batch):
        xh, xl = _split8(np.ascontiguousarray(x[b].T))
        xsp.append((_x_layout(xh), _x_layout(xl)))

    in_maps = []
    for c in range(8):
        b, g = c // 4, c % 4
        hs = slice(4 * g, 4 * g + 4)
        m = {"xh8": xsp[b][0], "xl8": xsp[b][1], "tri": tri,
             "z8": np.zeros((128, 2048), ml_dtypes.float8_e4m3)}
        for nm, W in (("wq", W_Q), ("wk", W_K), ("wv", W_V)):
            Wp = np.transpose(W[hs], (1, 0, 2)).reshape(dm, 256) * WS
            hi, lo = _split8(Wp)
            m[nm + "h"] = _w_layout(hi)
            m[nm + "l"] = _w_layout(lo)
        m["wo"] = np.ascontiguousarray(
            (W_O[hs].reshape(256, dm) / WS).astype(ml_dtypes.bfloat16))
        if with_bias:
            m["bqkv"] = np.ascontiguousarray(np.concatenate(
                [b_Q[hs].reshape(256) * WS, b_K[hs].reshape(256) * WS,
                 b_V[hs].reshape(256) * WS]
            )[None, :].astype(ml_dtypes.bfloat16))
            m["ones"] = np.ones((1, seq), ml_dtypes.bfloat16)
        in_maps.append(m)

    res = bass_utils.run_bass_kernel_spmd(nc, in_maps, core_ids=list(range(8)))
    parts = [res.results[c]["out"] for c in range(8)]
    full = np.stack(
        [parts[0] + parts[1] + parts[2] + parts[3],
         parts[4] + parts[5] + parts[6] + parts[7]]
    )
    full += b_O
    return full.astype(np.float32)


# revision 63
# speedup vs baseline: 1.0382x; 1.0173x over previous
"""Multi-head causal attention (B=2, S=2048, D=1024, H=16, Dh=64) on 8 TRN2 cores.

Sharding: tensor-parallel over 4 head-groups x data-parallel over 2 batches.
Core c handles batch c//4, heads [4*(c%4), 4*(c%4)+4). Each core computes its
partial output projection; the host sums the 4 partials per batch (the
"all-reduce") and adds b_O.

Mixed-precision device program (fp32 PSUM accumulation everywhere):
  - Q/K/V projections: fp8e4m3 DoubleRow matmuls (0.5 cyc/row, 256-deep
    contraction per instr). Host pre-splits x and 64*W into hi/lo fp8 pairs;
    3 compensation terms xh@Wh + xl@Wh + xh@Wl recover ~bf16 accuracy.
  - Scores S = (64q)(64k): bf16 QT/KT, per-128-column causal crops. The /8
    softmax scale and the 1/4096 fp8 weight scaling fold into the ACT exp
    (scale=1/32768).
  - P = exp(S): ACT writes fp8 tiles for off-diagonal k-pairs, bf16 for the
    4 diagonal k-tiles (where P concentrates and fp8 noise would not average
    out). Triangular masks multiply bf16 diag tiles only (DVE 2x on bf16).
  - Z = P @ [64V|1]: off-diag via fp8 DoubleRow over k-tile pairs (0.25
    cyc/row/k-tile), diag via bf16-moving matmuls; row 64 accumulates the
    softmax denominator.
  - out += (Z/denom)^T @ (Wo/64): f32r stationary ZT, bf16 moving WO.
"""

import numpy as np
import ml_dtypes

import concourse.mybir as mybir
import concourse.tile as tile
from concourse import bacc
from concourse import bass_utils

F32 = mybir.dt.float32
F32R = mybir.dt.float32r
BF16 = mybir.dt.bfloat16
F8 = mybir.dt.float8e4

SEQ = 2048
DM = 1024
DH = 64
HLOC = 4          # heads per core
KCP = 4           # dmodel pair-chunks of 256
NQC = 4           # q chunks of 512
QW = 512
WS = 64.0         # fp8 weight scale
EXP_SCALE = 1.0 / (WS * WS * 8.0)
DR = mybir.MatmulPerfMode.DoubleRow

_PROGRAMS = {}


def _build(with_bias: bool):
    nc = bacc.Bacc("TRN2", target_bir_lowering=False, debug=False, num_devices=8)

    # [128, j(4), i(2), qb(4), 512] with element [p,j,i,qb,s] = xT[256j+128i+p,
    # 512qb+s]; hi/lo fp8 split of xT
    xh8 = nc.dram_tensor("xh8", [128, KCP, 2, NQC, QW], F8, kind="ExternalInput").ap()
    xl8 = nc.dram_tensor("xl8", [128, KCP, 2, NQC, QW], F8, kind="ExternalInput").ap()
    # [128, j(4), i(2), 256]: hi/lo of 64*W[256j+128i+p, m]
    wname = lambda n: nc.dram_tensor(n, [128, KCP, 2, 256], F8, kind="ExternalInput").ap()
    wqh, wql = wname("wqh"), wname("wql")
    wkh, wkl = wname("wkh"), wname("wkl")
    wvh, wvl = wname("wvh"), wname("wvl")
    wo = nc.dram_tensor("wo", [256, DM], BF16, kind="ExternalInput").ap()
    z8 = nc.dram_tensor("z8", [128, 2048], F8, kind="ExternalInput").ap()
    tri = nc.dram_tensor("tri", [128, 128], BF16, kind="ExternalInput").ap()
    if with_bias:
        bqkv = nc.dram_tensor("bqkv", [1, 768], BF16, kind="ExternalInput").ap()
        ones = nc.dram_tensor("ones", [1, SEQ], BF16, kind="ExternalInput").ap()
    out = nc.dram_tensor("out", [SEQ, DM], F32, kind="ExternalOutput").ap()

    with tile.TileContext(nc) as tc:
        with (
            tc.tile_pool(name="px", bufs=1) as px,
            tc.tile_pool(name="pw", bufs=1) as pw,
            tc.tile_pool(name="pqk", bufs=1) as pqk,
            tc.tile_pool(name="pv", bufs=1) as pv,
            tc.tile_pool(name="ppt8", bufs=8) as ppt8,
            tc.tile_pool(name="pptb", bufs=4) as pptb,
            tc.tile_pool(name="pzt", bufs=4) as pzt,
            tc.tile_pool(name="prs", bufs=3) as prs,
            tc.tile_pool(name="pout", bufs=4) as pout,
            tc.tile_pool(name="psS", bufs=2, space="PSUM") as psS,
            tc.tile_pool(name="psZ", bufs=2, space="PSUM") as psZ,
            tc.tile_pool(name="psF", bufs=2, space="PSUM") as psF,
        ):
            # ---- SBUF tiles ----
            XH = px.tile([128, KCP, 2, NQC, QW], F8, tag="xh")
            XL = px.tile([128, KCP, 2, NQC, QW], F8, tag="xl")
            WQh = pw.tile([128, KCP, 2, 256], F8, tag="wqh")
            WQl = pw.tile([128, KCP, 2, 256], F8, tag="wql")
            WKh = pw.tile([128, KCP, 2, 256], F8, tag="wkh")
            WKl = pw.tile([128, KCP, 2, 256], F8, tag="wkl")
            WVh = pw.tile([128, KCP, 2, 256], F8, tag="wvh")
            WVl = pw.tile([128, KCP, 2, 256], F8, tag="wvl")

            QT = [pqk.tile([128, NQC, 2, QW], F8, tag=f"qt{hp}", name=f"qt{hp}")
                  for hp in range(2)]
            KT = [pqk.tile([128, 16, 2, 128], F8, tag=f"kt{hp}", name=f"kt{hp}")
                  for hp in range(2)]

            # ---- input DMA: qb0 x-blocks + QK weights first ----
            nc.sync.dma_start(WQh[:], wqh)
            nc.sync.dma_start(WQl[:], wql)
            nc.sync.dma_start(XH[:, 0, :, 0, :], xh8[:, 0, :, 0, :])
            nc.sync.dma_start(XL[:, 0, :, 0, :], xl8[:, 0, :, 0, :])
            nc.sync.dma_start(XH[:, 1:4, :, 0, :], xh8[:, 1:4, :, 0, :])
            nc.sync.dma_start(XL[:, 1:4, :, 0, :], xl8[:, 1:4, :, 0, :])
            nc.sync.dma_start(WKh[:], wkh)
            nc.sync.dma_start(WKl[:], wkl)
            for hp in range(2):
                nc.sync.dma_start(
                    QT[hp][:, :, 1, :],
                    z8[:, 0:NQC * QW].rearrange("p (a b) -> p a b", a=NQC))
                nc.sync.dma_start(
                    KT[hp][:, :, 1, :],
                    z8[:, 0:2048].rearrange("p (a b) -> p a b", a=16))
            nc.sync.dma_start(WVh[:], wvh)
            nc.sync.dma_start(WVl[:], wvl)
            TRI = pw.tile([128, 128], BF16, tag="tri")
            nc.sync.dma_start(TRI[:], tri)
            nc.sync.dma_start(XH[:, :, :, 1, :], xh8[:, :, :, 1, :])
            nc.sync.dma_start(XL[:, :, :, 1, :], xl8[:, :, :, 1, :])
            WO = pw.tile([128, 2, DM], BF16, tag="wo")
            for hp in range(2):
                nc.sync.dma_start(WO[:, hp, :], wo[hp * 128:(hp + 1) * 128, :])
            for qb in range(2, NQC):
                nc.sync.dma_start(XH[:, :, :, qb, :], xh8[:, :, :, qb, :])
                nc.sync.dma_start(XL[:, :, :, qb, :], xl8[:, :, :, qb, :])
            if with_bias:
                BQKV = pw.tile([1, 768], BF16, tag="bqkv")
                nc.sync.dma_start(BQKV[:], bqkv)
                ONES = pw.tile([1, SEQ], BF16, tag="ones")
                nc.sync.dma_start(ONES[:], ones)

            # QT/KT fp8 with an interleaved zero slot: scores run as
            # zero-padded DoubleRow matmuls at 0.5 cyc/row. Slot 1 of both
            # stationary and moving operands is zeroed via DMA so the second
            # dual-row tile contributes exactly 0.
            VB = pv.tile([128, 16, HLOC, DH + 1], BF16, tag="vb")
            # innermost padded to 68 so the DoubleRow pair stride (4*68)
            # is 16-element aligned (s3_lw_dual_fp8 ISA restriction)
            V8 = pv.tile([128, 6, 2, HLOC, DH + 4], F8, tag="v8")
            nc.vector.memset(VB[:, :, :, DH:DH + 1], 1.0)
            nc.vector.tensor_copy(
                V8[:, :, :, :, DH:DH + 1].rearrange("p a b h o -> p (a b) h o"),
                VB[:, 0:12, :, DH:DH + 1],
            )

            # ---- projections: fp8 DoubleRow 3-term chains (generators
            # yielding after each matmul so fillers can interleave) ----
            def qk_chain_g(pp_slice, w_hi, w_lo, hp, qc, bias_off):
                for j in range(KCP):
                    for term, (w_sb, x_sb) in enumerate(
                        ((w_hi, XH), (w_hi, XL), (w_lo, XH))
                    ):
                        nc.tensor.matmul(
                            pp_slice,
                            w_sb[:, j, :, hp * 128:(hp + 1) * 128],
                            x_sb[:, j, :, qc, :],
                            start=(j == 0 and term == 0),
                            stop=(j == KCP - 1 and term == 2 and not with_bias),
                            perf_mode=DR,
                        )
                        yield
                if with_bias:
                    nc.tensor.matmul(
                        pp_slice,
                        BQKV[0:1, bias_off + hp * 128:bias_off + (hp + 1) * 128],
                        ONES[0:1, qc * QW:(qc + 1) * QW],
                        start=False, stop=True,
                    )
                    yield

            def v_chain_g(pp_slice, st):
                qb, s0 = st // 4, (st % 4) * 128
                for j in range(KCP):
                    for term, (x_sb, w_sb) in enumerate(
                        ((XH, WVh), (XL, WVh), (XH, WVl))
                    ):
                        nc.tensor.matmul(
                            pp_slice,
                            x_sb[:, j, :, qb, s0:s0 + 128],
                            w_sb[:, j, :, :],
                            start=(j == 0 and term == 0),
                            stop=(j == KCP - 1 and term == 2 and not with_bias),
                            perf_mode=DR,
                        )
                        yield
                if with_bias:
                    nc.tensor.matmul(
                        pp_slice,
                        ONES[0:1, st * 128:(st + 1) * 128],
                        BQKV[0:1, 512:768],
                        start=False, stop=True,
                    )
                    yield

            class Filler:
                """FIFO of PE micro-work generators; m-loops pull a few
                matmuls at a time so proj/O-proj work interleaves with the
                ACT-paced attention stream."""

                def __init__(self):
                    self.units = []
                    self.cur = None
                    self.cur_marker = None

                def add(self, marker, genfn):
                    self.units.append((marker, genfn))

                def pull(self, n):
                    while n > 0:
                        if self.cur is None:
                            if not self.units:
                                return
                            self.cur_marker, genfn = self.units.pop(0)
                            self.cur = genfn()
                        try:
                            next(self.cur)
                            n -= 1
                        except StopIteration:
                            self.cur = None

                def drain_marker(self, marker):
                    while (self.cur is not None and self.cur_marker == marker) \
                            or any(m == marker for m, _ in self.units):
                        self.pull(1)

            # ---- projection chain-tile emitters (PE filler units) ----
            def qk_single_g(qc, hp, which):
                # Q/K chain as a psF single, generator form
                w_hi, w_lo, dst, boff = (
                    (WQh, WQl, QT, 0) if which == "q" else (WKh, WKl, KT, 256))
                pz = psF.tile([128, QW], F32, tag="f",
                              name=f"f{which}{qc}_{hp}")
                yield from qk_chain_g(pz[:], w_hi, w_lo, hp, qc, boff)
                eng = nc.scalar if qc <= 1 else nc.vector
                if which == "q":
                    eng.copy(dst[hp][:, qc, 0, :], pz[:]) if qc <= 1 else \
                        nc.vector.tensor_copy(dst[hp][:, qc, 0, :], pz[:])
                else:
                    if qc <= 1:
                        eng.copy(dst[hp][:, 4 * qc:4 * qc + 4, 0, :],
                                 pz[:].rearrange("p (k c) -> p k c", k=4))
                    else:
                        nc.vector.tensor_copy(
                            dst[hp][:, 4 * qc:4 * qc + 4, 0, :],
                            pz[:].rearrange("p (k c) -> p k c", k=4))

            def v_single_g(st):
                pz = psF.tile([128, QW], F32, tag="f", name=f"fv{st}")
                yield from v_chain_g(pz[:, 0:256], st)
                veng = nc.vector.tensor_copy if True else nc.scalar.copy
                veng(
                    VB[:, st, :, 0:DH],
                    pz[:, 0:256].rearrange("p (h d) -> p h d", h=HLOC),
                )
                if st < 12:
                    veng(
                        V8[:, st // 2, st % 2, :, 0:DH],
                        pz[:, 0:256].rearrange("p (h d) -> p h d", h=HLOC),
                    )

            def emit_qk_pair(qc, which):
                w_hi, w_lo, dst, boff = (
                    (WQh, WQl, QT, 0) if which == "q" else (WKh, WKl, KT, 256))
                pp = psS.tile([128, 2, QW], F32, tag="s", name=f"{which}p{qc}")
                for hp in range(2):
                    for _ in qk_chain_g(pp[:, hp, :], w_hi, w_lo, hp, qc, boff):
                        pass
                for hp in range(2):
                    if which == "q":
                        nc.scalar.copy(dst[hp][:, qc, 0, :], pp[:, hp, :])
                    else:
                        nc.scalar.copy(
                            dst[hp][:, 4 * qc:4 * qc + 4, 0, :],
                            pp[:, hp, :].rearrange("p (k c) -> p k c", k=4))

            def emit_v_pair(st2):
                pp = psS.tile([128, 2, QW], F32, tag="s", name=f"vp{st2}")
                for i in range(2):
                    for _ in v_chain_g(pp[:, i, 0:256], st2 * 2 + i):
                        pass
                for i in range(2):
                    st = st2 * 2 + i
                    nc.vector.tensor_copy(
                        VB[:, st, :, 0:DH],
                        pp[:, i, 0:256].rearrange("p (h d) -> p h d", h=HLOC),
                    )
                    if st < 12:
                        nc.vector.tensor_copy(
                            V8[:, st // 2, st % 2, :, 0:DH],
                            pp[:, i, 0:256].rearrange("p (h d) -> p h d", h=HLOC),
                        )

            # ---- output projection units (PE filler; one (qt,mc) each) ----
            ZTS = {}   # qc -> [ZT_hp0, ZT_hp1]
            OSB = {}   # (qc, qt) -> staging tile; DMA fires after mc=1

            def o_single_g(qc, qt, mc):
                q0 = qc * QW
                ZT = ZTS[qc]
                t = psF.tile([128, QW], F32, tag="f", name=f"o{qc}_{qt}_{mc}")
                if mc == 0:
                    OSB[(qc, qt)] = pout.tile([128, DM], F32, tag="ob",
                                              name=f"ob{qc}_{qt}")
                osb = OSB[(qc, qt)]
                for hp in range(2):
                    nc.tensor.matmul(
                        t[:],
                        ZT[hp][:, qt * 128:(qt + 1) * 128],
                        WO[:, hp, mc * QW:(mc + 1) * QW],
                        start=(hp == 0), stop=(hp == 1),
                    )
                    yield
                if qc == 3 and (qt + mc) % 2 == 1:
                    # tail: ACT is idle after the last exp; splitting the
                    # final copies across DVE+ACT halves the drain
                    nc.scalar.copy(osb[:, mc * QW:(mc + 1) * QW], t[:])
                else:
                    nc.vector.tensor_copy(osb[:, mc * QW:(mc + 1) * QW], t[:])
                if qc == 3:
                    nc.sync.dma_start(
                        out[q0 + qt * 128:q0 + (qt + 1) * 128,
                            mc * QW:(mc + 1) * QW],
                        osb[:, mc * QW:(mc + 1) * QW])
                elif mc == 1:
                    nc.sync.dma_start(
                        out[q0 + qt * 128:q0 + (qt + 1) * 128, :], osb[:])

            def emit_o_unit(qc, qt):
                for mc in range(2):
                    for _ in o_single_g(qc, qt, mc):
                        pass

            # ---- one attention head: scores->exp->Z with the Z matmuls
            # delayed one s-tile behind (exp latency hiding) and filler
            # matmuls pulled between iterations ----
            norm_pend = []   # deferred (bcast-mm + ZT-mul) closures

            def drain_norm():
                while norm_pend:
                    norm_pend.pop(0)()

            def emit_head(qc, h, filler):
                hp, hh = h // 2, h % 2
                hs = slice(hh * DH, (hh + 1) * DH)
                zps = psZ.tile([128, QW], F32, tag="z", name=f"z{qc}_{h}")

                def s_mm(dst, kt, q_lo, q_hi):
                    nc.tensor.matmul(
                        dst,
                        KT[hp][hs, kt, :, :],
                        QT[hp][hs, qc, :, q_lo:q_hi],
                        start=True, stop=True,
                        perf_mode=DR,
                    )

                def z_dr(m, pt8):
                    nc.tensor.matmul(
                        zps[0:DH + 1, :], V8[:, m, :, h, 0:DH + 1], pt8[:],
                        start=(m == 0), stop=False,
                        perf_mode=DR, skip_group_check=True,
                    )

                pend = []
                for m in range(2 * qc):
                    sps = psS.tile([128, 2, QW], F32, tag="s",
                                   name=f"s{qc}_{h}_{m}")
                    for i in range(2):
                        s_mm(sps[:, i, :], 2 * m + i, 0, QW)
                    drain_norm()  # deferred prev-head normalize on PE slack
                    pt8 = ppt8.tile([128, 2, QW], F8, tag="p8",
                                    name=f"p8_{qc}_{h}_{m}")
                    nc.scalar.activation(
                        pt8[:], sps[:], mybir.ActivationFunctionType.Exp,
                        scale=EXP_SCALE,
                    )
                    pend.append((z_dr, m, pt8))
                    filler.pull(3)
                    if len(pend) > 1:
                        fn, mm, pt = pend.pop(0)
                        fn(mm, pt)

                ktA, ktB = 4 * qc, 4 * qc + 2
                sA = psS.tile([128, 2, QW], F32, tag="s", name=f"sA{qc}_{h}")
                s_mm(sA[:, 0, :], ktA, 0, QW)
                s_mm(sA[:, 1, :], ktA + 1, 0, QW)
                drain_norm()
                ptA = pptb.tile([128, 2, QW], BF16, tag="pb", name=f"pA{qc}_{h}")
                nc.scalar.activation(
                    ptA[:], sA[:], mybir.ActivationFunctionType.Exp,
                    scale=EXP_SCALE,
                )
                for fn, mm, pt in pend:
                    fn(mm, pt)
                sB = psS.tile([128, 2, QW], F32, tag="s", name=f"sB{qc}_{h}")
                s_mm(sB[:, 0, 0:256], ktB, 256, QW)
                s_mm(sB[:, 0, 256:QW], ktB + 1, 256, QW)
                ptB = pptb.tile([128, 2, QW], BF16, tag="pb", name=f"pB{qc}_{h}")
                nc.scalar.activation(
                    ptB[:, 0, :], sB[:, 0, :],
                    mybir.ActivationFunctionType.Exp, scale=EXP_SCALE,
                )
                meng = nc.vector if (qc == 3 and h == 3) else nc.gpsimd
                meng.tensor_mul(ptA[:, 0, 0:128], ptA[:, 0, 0:128], TRI[:])
                meng.tensor_mul(ptA[:, 1, 128:256], ptA[:, 1, 128:256], TRI[:])
                meng.tensor_mul(ptB[:, 0, 0:128], ptB[:, 0, 0:128], TRI[:])
                meng.tensor_mul(ptB[:, 0, 384:QW], ptB[:, 0, 384:QW], TRI[:])

                # PE filler while the diagonal exps+masks complete
                filler.pull(4)

                def z_bf(v_st, pt_ap, q_lo, q_hi, start, stop):
                    nc.tensor.matmul(
                        zps[0:DH + 1, q_lo:q_hi],
                        VB[:, v_st, h, :], pt_ap,
                        start=start, stop=stop, skip_group_check=True,
                    )

                z_bf(ktA, ptA[:, 0, :], 0, QW, qc == 0, False)
                z_bf(ktA + 1, ptA[:, 1, 128:QW], 128, QW, False, False)
                z_bf(ktB, ptB[:, 0, 0:256], 256, QW, False, False)
                z_bf(ktB + 1, ptB[:, 0, 384:QW], 384, QW, False, True)

                recip = prs.tile([1, QW], F32R, tag="recip", name=f"rc{qc}_{h}")
                with nc.allow_low_precision(reason="softmax recip in fp32r"):
                    nc.vector.reciprocal(recip[:], zps[DH:DH + 1, :])
                rb = prs.tile([DH, QW], F32R, tag="rb", name=f"rb{qc}_{h}")
                nc.gpsimd.partition_broadcast(rb[:], recip[:])
                if qc not in ZTS:
                    ZTS[qc] = [
                        pzt.tile([128, QW], BF16, tag="zt", name=f"zt{qc}_{p}")
                        for p in range(2)
                    ]
                nc.vector.tensor_mul(ZTS[qc][hp][hs, :], zps[0:DH, :], rb[:])

            # ---- global pipelined emission ----
            # minimal prefix so the first exp fires early: Q-hp0, K pair,
            # V pairs for qc0 diag; everything else rides the filler queue
            for _ in qk_single_g(0, 0, "q"):
                pass
            emit_qk_pair(0, "k")
            for st in range(4):
                for _ in v_single_g(st):
                    pass

            filler = Filler()
            filler.add("p0", lambda: qk_single_g(0, 1, "q"))
            for qcn in (1, 2, 3):
                for which in ("q", "k"):
                    for hp in range(2):
                        filler.add(f"p{qcn}",
                                   (lambda qcn=qcn, hp=hp, which=which:
                                    qk_single_g(qcn, hp, which)))
                for st in range(4 * qcn, 4 * qcn + 4):
                    filler.add(f"p{qcn}", lambda st=st: v_single_g(st))
            def add_o_units(qcp):
                # added only after head(qcp,3) is emitted so no O matmul can
                # be pulled ahead of its ZT writes
                for qt in range(4):
                    for mc in range(2):
                        filler.add(f"o{qcp}",
                                   (lambda qcp=qcp, qt=qt, mc=mc:
                                    o_single_g(qcp, qt, mc)))

            # head order: qc-major, but (3,0) hoisted before (2,3) so qc3's
            # ACT work starts earlier
            order = [(0, 0), (0, 1), (0, 2), (0, 3),
                     (1, 0), (1, 1), (1, 2), (1, 3),
                     (2, 0), (2, 1), (2, 2), (3, 0), (2, 3),
                     (3, 1), (3, 2), (3, 3)]
            for qc, h in order:
                if h == 0:
                    filler.drain_marker(f"p{qc}")
                if qc == 0 and h == 2:
                    filler.drain_marker("p0")  # QT hp1 needed
                emit_head(qc, h, filler)
                if h == 3 and qc < 3:
                    add_o_units(qc)
            drain_norm()
            filler.drain_marker("o2")
            for qt in range(4):
                emit_o_unit(3, qt)

    nc.compile()
    return nc


def _get_program(with_bias: bool):
    if with_bias not in _PROGRAMS:
        _PROGRAMS[with_bias] = _build(with_bias)
    return _PROGRAMS[with_bias]


def _split8(a):
    hi = a.astype(ml_dtypes.float8_e4m3)
    lo = (a - hi.astype(np.float32)).astype(ml_dtypes.float8_e4m3)
    return hi, lo


def _x_layout(a):
    # [1024, 2048] -> [128, j, i, qb, s]
    return np.ascontiguousarray(
        a.reshape(KCP, 2, 128, NQC, QW).transpose(2, 0, 1, 3, 4))


def _w_layout(a):
    # [1024, 256] -> [128, j, i, 256]
    return np.ascontiguousarray(
        a.reshape(KCP, 2, 128, 256).transpose(2, 0, 1, 3))


def kernel(normalized_resid_pre, W_Q, W_K, W_V, W_O, b_Q, b_K, b_V, b_O):
    x = np.asarray(normalized_resid_pre, dtype=np.float32)
    W_Q = np.asarray(W_Q, dtype=np.float32)
    W_K = np.asarray(W_K, dtype=np.float32)
    W_V = np.asarray(W_V, dtype=np.float32)
    W_O = np.asarray(W_O, dtype=np.float32)
    b_Q = np.asarray(b_Q, dtype=np.float32)
    b_K = np.asarray(b_K, dtype=np.float32)
    b_V = np.asarray(b_V, dtype=np.float32)
    b_O = np.asarray(b_O, dtype=np.float32)

    batch, seq, dm = x.shape
    with_bias = bool(np.any(b_Q) or np.any(b_K) or np.any(b_V))
    nc = _get_program(with_bias)

    tri = np.ascontiguousarray(
        np.triu(np.ones((128, 128), np.float32)).astype(ml_dtypes.bfloat16))

    xsp = []
    for b in range(batch):
        xh, xl = _split8(np.ascontiguousarray(x[b].T))
        xsp.append((_x_layout(xh), _x_layout(xl)))

    in_maps = []
    for c in range(8):
        b, g = c // 4, c % 4
        hs = slice(4 * g, 4 * g + 4)
        m = {"xh8": xsp[b][0], "xl8": xsp[b][1], "tri": tri,
             "z8": np.zeros((128, 2048), ml_dtypes.float8_e4m3)}
        for nm, W in (("wq", W_Q), ("wk", W_K), ("wv", W_V)):
            Wp = np.transpose(W[hs], (1, 0, 2)).reshape(dm, 256) * WS
            hi, lo = _split8(Wp)
            m[nm + "h"] = _w_layout(hi)
            m[nm + "l"] = _w_layout(lo)
        m["wo"] = np.ascontiguousarray(
            (W_O[hs].reshape(256, dm) / WS).astype(ml_dtypes.bfloat16))
        if with_bias:
            m["bqkv"] = np.ascontiguousarray(np.concatenate(
                [b_Q[hs].reshape(256) * WS, b_K[hs].reshape(256) * WS,
                 b_V[hs].reshape(256) * WS]
            )[None, :].astype(ml_dtypes.bfloat16))
            m["ones"] = np.ones((1, seq), ml_dtypes.bfloat16)
        in_maps.append(m)

    res = bass_utils.run_bass_kernel_spmd(nc, in_maps, core_ids=list(range(8)))
    parts = [res.results[c]["out"] for c in range(8)]
    full = np.stack(
        [parts[0] + parts[1] + parts[2] + parts[3],
         parts[4] + parts[5] + parts[6] + parts[7]]
    )
    full += b_O
    return full.astype(np.float32)
